# revision 1
# baseline (speedup 1.0000x reference)
"""Grouped-experts SwiGLU FFN (MoE) on 8 Trainium2 NeuronCores.

Expert-parallel: core e owns expert e's weights and its contiguous token
slice.  Tokens are already sorted by expert (contiguous ranges from
cumsum(num_tokens_per_expert)), so the "all-to-all dispatch" is plain host
slicing.  Each core runs a two-stage SwiGLU:

  stage 1:  HT[h, t] = silu(W1 x)[h, t] * (W3 x)[h, t]     (K = DIM)
  stage 2:  OUT.T[d, t] = (W2 @ H)[d, t]                   (K = HIDDEN)

Matmuls run in bf16 (1 cycle/row on the PE array, fp32 PSUM accumulate);
~4e-3 relative error vs the fp32 reference.  Host pre-packs x and weights
into SBUF-tile layout [128, ...] so every DMA reads contiguous lines; the
kernel returns OUT.T per core and the host transposes/scatters back.

fp8 DoubleRow (2x PE throughput) everywhere was evaluated and rejected:
e4m3's 3-bit mantissa gives ~2.6% per-element quant error and the
three-matmul chain lands at ~6.5% rel err vs the 2e-2 budget.  But a
SURGICAL fp8 slice fits: 2 of stage 2's 11 contraction blocks run as one
DoubleRow matmul (512 streaming cycles -> ~578/2), saving ~60ns per
output tile; with the bf16 output store the measured rel err is
1.644e-2 < 2e-2.  Scales are
folded into host weight packing (w3 blocks 0,1 x4 so the DVE mul emits
e4m3(4h); w3 blocks 2..10 x128 so bf16 ht carries 128h; fp8 w2 blocks
x32 so both PSUM partials are 128x the true value; one scaled Copy
drains /128).
"""

import numpy as np
import ml_dtypes

import concourse.bass as bass
from concourse import bacc
import concourse.mybir as mybir
from concourse.tile import TileContext
from concourse.bass_utils import run_bass_kernel_spmd

N_TOKENS = 16384
DIM = 2048
HIDDEN = 1408
N_EXPERTS = 8
N_CORES = 8

P = 128
T = 2048                 # token capacity per core per pass
N_DN = DIM // P          # 16 contraction blocks in stage 1
N_HT = HIDDEN // P       # 11 h tiles
N_F8 = 2                 # h-blocks 0..1 run stage 2 in fp8 DoubleRow
N_HB = N_HT - N_F8       # h-blocks 2..10 run stage 2 in bf16
N_DT = DIM // P          # 16 output-row tiles in stage 2
TSUB = 512               # moving-operand width per matmul (1 PSUM bank)

F32 = mybir.dt.float32
BF16 = mybir.dt.bfloat16
F8E4 = mybir.dt.float8e4
SILU = mybir.ActivationFunctionType.Silu
COPY = mybir.ActivationFunctionType.Copy
DROW = mybir.MatmulPerfMode.DoubleRow
BF = ml_dtypes.bfloat16
F8 = ml_dtypes.float8_e4m3
H8_SCALE = 4.0           # ht8 = e4m3(4h); |4h|max ~62 << 240
W28_SCALE = 32.0         # w2 fp8 blocks x32; |32 w2|max ~3.5
HT_SCALE = 128.0         # bf16 ht carries 128h; PSUM holds 128*out


def _build_program() -> bass.Bass:
    nc = bacc.Bacc()
    xtp = nc.declare_dram_parameter(
        "xtp", [P, T // TSUB, N_DN, TSUB], BF16, isOutput=False)
    w1p = nc.declare_dram_parameter("w1p", [P, N_HT, N_DN, P], BF16, isOutput=False)
    w3p = nc.declare_dram_parameter("w3p", [P, N_HT, N_DN, P], BF16, isOutput=False)
    w2p = nc.declare_dram_parameter("w2p", [P, N_DT, N_HB, P], BF16, isOutput=False)
    w28p = nc.declare_dram_parameter("w28p", [P, N_DT, N_F8, P], F8E4, isOutput=False)
    # output in bf16: halves drain-DMA bytes and the exposed final
    # transfer; adds ~0.17% per-element quant on top of the 1.64e-2 fp8
    # error (host upcasts back to f32)
    outt = nc.declare_dram_parameter("outt", [DIM, T], BF16, isOutput=True)

    with TileContext(nc) as tc:
        with (
            tc.tile_pool(name="xt", bufs=1) as xt_pool,
            tc.tile_pool(name="ht", bufs=1) as ht_pool,
            tc.tile_pool(name="w1", bufs=3) as w1_pool,
            tc.tile_pool(name="w3", bufs=3) as w3_pool,
            tc.tile_pool(name="w2", bufs=1) as w2_pool,
            tc.tile_pool(name="w28", bufs=1) as w28_pool,
            tc.tile_pool(name="tmp", bufs=3) as tmp_pool,
            tc.tile_pool(name="ob", bufs=2) as ob_pool,
            tc.tile_pool(name="ps", bufs=1, space="PSUM") as ps_pool,
        ):
            # DMA queue order = program order: quarter-loads of
            # (w1, x-chunk0, w3) first so the first matmuls' operands land
            # first, then the later x chunks also in quarters -- the PE
            # consumes them progressively (a whole-chunk load measurably
            # stalls the PE ~3us waiting for the full 2MB)
            w1b0 = w1_pool.tile([P, N_DN, P], BF16, tag="w1")
            w3b0 = w3_pool.tile([P, N_DN, P], BF16, tag="w3")
            xcs = [
                xt_pool.tile([P, N_DN, TSUB], BF16, bufs=T // TSUB,
                             tag="xt", name=f"xc{i}")
                for i in range(T // TSUB)
            ]
            # All bulk loads stay on the Sync HW-DGE ring.  (Tried: x loads
            # on the Scalar engine's qActDynamicHW ring to parallelize the
            # ~0.6us/dma_start descriptor programming -- the first matmul
            # did start ~1-3us earlier, but the Act ring sustains far less
            # bandwidth than the SP ring, starving the later x chunks for
            # a net loss: xc0-only +0.5us, all-x +15us.)
            # Loads are in exact PE-consumption order: the w1 pass (MMs
            # 1-16) reads w1q(n/4)+xq(n/4) alternately; w3 is only needed
            # from MM 17, so its 1MB comes after -- the head is delivery-
            # bound and out-of-order delivery just lengthens the ramp.
            for q in range(4):
                dn = slice(q * 4, (q + 1) * 4)
                nc.sync.dma_start(out=w1b0[:, dn, :], in_=w1p[:, 0, dn, :])
                nc.sync.dma_start(out=xcs[0][:, dn, :], in_=xtp[:, 0, dn, :])
            for q in range(4):
                dn = slice(q * 4, (q + 1) * 4)
                nc.sync.dma_start(out=w3b0[:, dn, :], in_=w3p[:, 0, dn, :])
            for c in range(1, T // TSUB):
                for q in range(4):
                    dn = slice(q * 4, (q + 1) * 4)
                    nc.sync.dma_start(out=xcs[c][:, dn, :],
                                      in_=xtp[:, c, dn, :])
            xts = xcs
            ht = ht_pool.tile([P, N_HB, T], BF16)
            ht8 = ht_pool.tile([P, N_F8, T], F8E4, name="ht8")

            # stage 1: HT[h, t] = silu(x @ w1.T).T * (x @ w3.T).T
            # (host pre-scaled w3 so blocks 0..1 emit e4m3(4h) and blocks
            # 2..10 emit bf16(128h) straight out of the DVE mul)
            for ih in range(N_HT):
                if ih == 0:
                    w1b, w3b = w1b0, w3b0
                else:
                    w1b = w1_pool.tile([P, N_DN, P], BF16, tag="w1")
                    nc.sync.dma_start(out=w1b[:], in_=w1p[:, ih, :, :])
                    w3b = w3_pool.tile([P, N_DN, P], BF16, tag="w3")
                    nc.sync.dma_start(out=w3b[:], in_=w3p[:, ih, :, :])
                for its in range(T // TSUB):
                    ts0 = its * TSUB
                    xt_c = xts[its]
                    ps1 = ps_pool.tile([P, TSUB], F32, bufs=2, name="ps1")
                    ps2 = ps_pool.tile([P, TSUB], F32, bufs=2, name="ps2")
                    for n in range(N_DN):
                        nc.tensor.matmul(
                            ps1[:],
                            lhsT=w1b[:, n, :],
                            rhs=xt_c[:, n, :],
                            start=(n == 0),
                            stop=(n == N_DN - 1),
                        )
                    for n in range(N_DN):
                        nc.tensor.matmul(
                            ps2[:],
                            lhsT=w3b[:, n, :],
                            rhs=xt_c[:, n, :],
                            start=(n == 0),
                            stop=(n == N_DN - 1),
                        )
                    tmp = tmp_pool.tile([P, TSUB], F32)
                    nc.scalar.activation(tmp[:], ps1[:], SILU)
                    if ih < N_F8:
                        dst = ht8[:, ih, ts0:ts0 + TSUB]
                    else:
                        dst = ht[:, ih - N_F8, ts0:ts0 + TSUB]
                    nc.vector.tensor_mul(dst, tmp[:], ps2[:])

            # stage 2: OUT.T[d, t] = sum_h W2T[h, d] * HT[h, t]
            # h-blocks 0..1 as one fp8 DoubleRow matmul per (idt, its)
            # (PSUM partial is (4h)*(32 w2) = 128*true, matching the bf16
            # partials (128h)*w2); drain divides by 128.  All w2 weights
            # load as 2 whole-tensor DMAs (SBUF has room; kills 32 of the
            # ~0.6us dma_start programming slots on the Sync engine).
            w2s = w2_pool.tile([P, N_DT, N_HB, P], BF16)
            nc.sync.dma_start(out=w2s[:], in_=w2p[:])
            w28s = w28_pool.tile([P, N_DT, N_F8, P], F8E4)
            nc.sync.dma_start(out=w28s[:], in_=w28p[:])
            NTS = T // TSUB
            for idt in range(N_DT):
                # ts-major: one PSUM bank accumulates DR + 9 bf16, drains
                # via a scaled Copy on ScalarE while the next chunk streams;
                # bank rotation happens once per 10 matmuls (per-MM rotation
                # measurably costs ~2.5us in PE micro-gaps)
                ob = ob_pool.tile([P, T], BF16)
                for its in range(NTS):
                    seg = slice(its * TSUB, (its + 1) * TSUB)
                    pso = ps_pool.tile([P, TSUB], F32, bufs=4, name="pso")
                    nc.tensor.matmul(
                        pso[:],
                        lhsT=w28s[:, idt, :, :],
                        rhs=ht8[:, :, seg],
                        start=True,
                        stop=False,
                        perf_mode=DROW,
                    )
                    for hb in range(N_HB):
                        nc.tensor.matmul(
                            pso[:],
                            lhsT=w2s[:, idt, hb, :],
                            rhs=ht[:, hb, seg],
                            start=False,
                            stop=(hb == N_HB - 1),
                        )
                    if idt < N_DT - 1:
                        nc.scalar.activation(ob[:, seg], pso[:],
                                             COPY, scale=1.0 / HT_SCALE)
                    elif its < NTS - 1:
                        # last tile: chunks drain individually right away
                        # so only the final chunk remains after the last MM
                        nc.scalar.activation(ob[:, seg], pso[:],
                                             COPY, scale=1.0 / HT_SCALE)
                        nc.sync.dma_start(
                            out=outt[idt * P:(idt + 1) * P, seg],
                            in_=ob[:, seg])
                    else:
                        # final chunk split in two so the exposed tail is
                        # only act(256) + program + dma(256)
                        for h in range(2):
                            lo = its * TSUB + h * (TSUB // 2)
                            hi = lo + TSUB // 2
                            nc.scalar.activation(
                                ob[:, lo:hi],
                                pso[:, h * (TSUB // 2):(h + 1) * (TSUB // 2)],
                                COPY, scale=1.0 / HT_SCALE)
                            nc.sync.dma_start(
                                out=outt[idt * P:(idt + 1) * P, lo:hi],
                                in_=ob[:, lo:hi])
                if idt < N_DT - 1:
                    # one batched drain DMA per output-row tile
                    nc.sync.dma_start(out=outt[idt * P:(idt + 1) * P, :],
                                      in_=ob[:])
    nc.compile()
    return nc


_CACHE: dict = {}


def _get_nc() -> bass.Bass:
    if "nc" not in _CACHE:
        _CACHE["nc"] = _build_program()
    return _CACHE["nc"]


def _pack_weights(w1, w2, w3):
    maps = []
    for e in range(N_EXPERTS):
        w3s = w3[e].copy()
        w3s[:N_F8 * P] *= H8_SCALE
        w3s[N_F8 * P:] *= HT_SCALE
        w2bf = w2[e][:, N_F8 * P:]
        w28 = np.clip(w2[e][:, :N_F8 * P] * W28_SCALE, -240.0, 240.0)
        maps.append({
            "w1p": np.ascontiguousarray(
                w1[e].reshape(N_HT, P, N_DN, P).transpose(3, 0, 2, 1).astype(BF)),
            "w3p": np.ascontiguousarray(
                w3s.reshape(N_HT, P, N_DN, P).transpose(3, 0, 2, 1).astype(BF)),
            "w2p": np.ascontiguousarray(
                w2bf.reshape(N_DT, P, N_HB, P).transpose(3, 0, 2, 1).astype(BF)),
            "w28p": np.ascontiguousarray(
                w28.reshape(N_DT, P, N_F8, P).transpose(3, 0, 2, 1).astype(F8)),
        })
    return maps


def kernel(x, w1, w2, w3, num_tokens_per_expert, _trace=False):
    x = np.ascontiguousarray(np.asarray(x, dtype=np.float32))
    w1 = np.ascontiguousarray(np.asarray(w1, dtype=np.float32))
    w2 = np.ascontiguousarray(np.asarray(w2, dtype=np.float32))
    w3 = np.ascontiguousarray(np.asarray(w3, dtype=np.float32))
    counts = np.asarray(num_tokens_per_expert, dtype=np.int64)

    cs = np.cumsum(counts)
    starts = np.minimum(np.concatenate([[0], cs[:-1]]), N_TOKENS)
    ends = np.minimum(cs, N_TOKENS)
    lens = np.maximum(ends - starts, 0)

    wmaps = _pack_weights(w1, w2, w3)
    out = np.zeros((N_TOKENS, DIM), np.float32)
    trace_info = []

    n_passes = max(1, int(np.max(np.ceil(lens / T))))
    for k in range(n_passes):
        in_maps = []
        for e in range(N_EXPERTS):
            s = int(starts[e]) + k * T
            xe = np.zeros((T, DIM), np.float32)
            avail = x[s:s + T]
            if avail.shape[0]:
                xe[:avail.shape[0]] = avail
            # [P, n_chunks, N_DN, TSUB]: xtp[p, c, n, t] = x[c*TSUB+t, n*128+p]
            xtp = np.ascontiguousarray(
                xe.T.reshape(N_DN, P, T // TSUB, TSUB)
                .transpose(1, 2, 0, 3).astype(BF))
            in_maps.append({"xtp": xtp, **wmaps[e]})
        res = run_bass_kernel_spmd(
            _get_nc(), in_maps, list(range(N_CORES)), trace=_trace
        )
        if _trace:
            trace_info.append(res)
        for e in range(N_EXPERTS):
            s = int(starts[e]) + k * T
            cnt = min(int(ends[e]) - s, T)
            if cnt > 0:
                out[s:s + cnt] = res.results[e]["outt"].T[:cnt]

    if _trace:
        return out, trace_info
    return out



# revision 14
# speedup vs baseline: 1.0870x; 1.0870x over previous
"""Grouped-experts SwiGLU FFN (MoE) on 8 Trainium2 NeuronCores.

Expert-parallel: core e owns expert e's weights and its contiguous token
slice (tokens are pre-sorted by expert).  Per core, out.T = W2 @ h where
h = silu(W1 x) * (W3 x), x [2048 dim, 2048 tok].

Stage 1 runs one level of Strassen on the stacked [W1; W3] @ x product:
A = [W1; W3] is [2816, 2048] (M-halves of 1408 = 11 clean 128-row tiles),
K = 2048 and N = 2048 both split 1024.  7 M-products instead of 8
block-products cuts stage-1 PE streaming by 12.5% (1232 vs 1408 matmuls).
The A-side combinations are folded into host weight packing; the B-side
(x) sums are 5 DVE adds per token-column-pair; M-product drains and the
C recombination ride the otherwise-idle Vector engine under the PE
shadow.  silu(C_top) * C_bot then feeds stage 2 unchanged.

Precision: fp16 operands everywhere (same PE speed as bf16, 8x lower
noise), plus e4m3 fp8 DoubleRow (2 contraction blocks per matmul) for
stage-2 h-blocks 0,1 on all tokens and blocks 2,3 on tokens 0:1024.
Offline-simulated rel err 1.958e-2 vs the 2e-2 budget (HW measured
tracks the simulator within ~3e-4).  fp16 w2 is host-scaled x128 so its
PSUM partials match the (4h)(32w2) fp8 partials; drains scale by 1/128.

Head: the runtime preamble is ~7us; a few junk warm-up matmuls issued
first get the PE HAM clock-gate to 2.4 GHz before real operands land,
and the DMA descriptor order streams exactly what the first matmul
chain needs (A(m2) tiles + x chunk 0) at full bandwidth.
"""

import numpy as np
import ml_dtypes

import concourse.bass as bass
from concourse import bacc
import concourse.mybir as mybir
from concourse.tile import TileContext
from concourse.bass_utils import run_bass_kernel_spmd

N_TOKENS = 16384
DIM = 2048
HIDDEN = 1408
N_EXPERTS = 8
N_CORES = 8

P = 128
T = 2048                 # token capacity per core per pass
N_DN = DIM // P          # 16 k-blocks (full K)
HK = 8                   # k-blocks per Strassen half (1024/128)
N_MT = HIDDEN // P       # 11 output-row tiles per M-product
N_HT = N_MT              # 11 h blocks
N_F8F = 2                # h-blocks 0,1: fp8 DR for all tokens
N_F8H = 2                # h-blocks 2,3: fp8 DR for tokens 0:1024
N_HB = 7                 # h-blocks 4..10: fp16-only path
N_W2F = 9                # fp16 w2 blocks 2..10 (2,3 used for tokens 1024:2048)
N_DT = DIM // P          # 16 output-row tiles in stage 2
TSUB = 512
HALF = T // 2

F32 = mybir.dt.float32
F16 = mybir.dt.float16
F8E4 = mybir.dt.float8e4
SILU = mybir.ActivationFunctionType.Silu
COPY = mybir.ActivationFunctionType.Copy
DROW = mybir.MatmulPerfMode.DoubleRow
ADD = mybir.AluOpType.add
SUB = mybir.AluOpType.subtract
NPF16 = np.float16
F8 = ml_dtypes.float8_e4m3
H8_SCALE = 4.0           # ht8 = e4m3(4h)
W28_SCALE = 32.0         # fp8 w2 blocks x32 -> PSUM partial 128*out
W2F_SCALE = 128.0        # fp16 w2 x128 -> matches fp8 partial scale

# Strassen M-product index order within phase B (phase A runs m2 alone):
# m4 first (its T-sum needs only x chunk A), then m5 (raw), m1/m3/m6/m7.
M2 = 1                   # m-index (0-based) of M2 = (A21+A22) B11
PHASE_B = [3, 4, 0, 2, 5, 6]   # m4, m5, m1, m3, m6, m7
N_WU = 6                 # warm-up matmuls during the runtime preamble


def _build_program() -> bass.Bass:
    nc = bacc.Bacc()
    # A combinations: ap[p, mt, m, kb, c] = A_m[mt*128+c, kb*128+p]
    ap = nc.declare_dram_parameter("ap", [P, N_MT, 7, HK, P], F16, isOutput=False)
    xtp = nc.declare_dram_parameter(
        "xtp", [P, T // TSUB, N_DN, TSUB], F16, isOutput=False)
    w2p = nc.declare_dram_parameter("w2p", [P, N_DT, N_W2F, P], F16, isOutput=False)
    w28p = nc.declare_dram_parameter(
        "w28p", [P, N_DT, N_F8F + N_F8H, P], F8E4, isOutput=False)
    outt = nc.declare_dram_parameter("outt", [DIM, T], F16, isOutput=True)

    with TileContext(nc) as tc:
        with (
            tc.tile_pool(name="wu", bufs=1) as wu_pool,
            tc.tile_pool(name="xt", bufs=1) as xt_pool,
            tc.tile_pool(name="at", bufs=5) as at_pool,
            tc.tile_pool(name="tt", bufs=1) as tt_pool,
            tc.tile_pool(name="m2s", bufs=1) as m2s_pool,
            tc.tile_pool(name="ms", bufs=2) as ms_pool,
            tc.tile_pool(name="cc", bufs=1) as cc_pool,
            tc.tile_pool(name="ht", bufs=1) as ht_pool,
            tc.tile_pool(name="w2", bufs=2) as w2_pool,
            tc.tile_pool(name="w28", bufs=2) as w28_pool,
            tc.tile_pool(name="tmp", bufs=4) as tmp_pool,
            tc.tile_pool(name="ob", bufs=2) as ob_pool,
            tc.tile_pool(name="ps", bufs=1, space="PSUM") as ps_pool,
        ):
            # ---- PE warm-up: junk matmuls issued before any DMA lands so
            # the HAM clock-gate reaches 2.4 GHz during the ~7us preamble.
            wu = wu_pool.tile([P, TSUB], F16)
            nc.vector.memset(wu[:], 0.0)
            for _ in range(N_WU):
                psw = ps_pool.tile([P, TSUB], F32, bufs=3, name="psm")
                nc.tensor.matmul(psw[:], lhsT=wu[:, 0:P], rhs=wu[:],
                                 start=True, stop=True)

            # ---- bulk input tiles: chunk pair (cA, cB) per column pass;
            # p=1 reuses p=0's buffers (WAR clears once p=0 stops reading)
            def chunk_tiles():
                cA = xt_pool.tile([P, N_DN, TSUB], F16, bufs=1, tag="xta",
                                  name="xcA")
                cB = xt_pool.tile([P, N_DN, TSUB], F16, bufs=1, tag="xtb",
                                  name="xcB")
                return cA, cB

            def load_a(mt, m):
                at = at_pool.tile([P, HK, P], F16, tag="at")
                nc.sync.dma_start(out=at[:], in_=ap[:, mt, m, :, :])
                return at

            # Head-critical DMA order: A(mt0,m2), x chunk0 (fine-grained so
            # the first matmul chain starts ASAP), more A(m2), x chunk2.
            # (only 4 A-tiles preloaded here -- the at-pool ring is 5 deep and
            # a blocked ring WAR would stall the sync FIFO behind it)
            cA0, cB0 = chunk_tiles()
            a2_tiles = {}
            a2_tiles[0] = load_a(0, M2)
            for qkb in range(0, HK, 2):
                nc.sync.dma_start(out=cA0[:, qkb:qkb + 2, :],
                                  in_=xtp[:, 0, qkb:qkb + 2, :])
            a2_tiles[1] = load_a(1, M2)
            nc.sync.dma_start(out=cA0[:, HK:, :], in_=xtp[:, 0, HK:, :])
            a2_tiles[2] = load_a(2, M2)
            a2_tiles[3] = load_a(3, M2)
            for h in range(2):
                nc.sync.dma_start(out=cB0[:, h * HK:(h + 1) * HK, :],
                                  in_=xtp[:, 2, h * HK:(h + 1) * HK, :])

            # ---- stage-1 outputs
            ht = ht_pool.tile([P, N_HB, T], F16)                 # blocks 4..10
            ht23 = ht_pool.tile([P, N_F8H, HALF], F16, name="ht23")  # 2,3 hi-half
            ht8f = ht_pool.tile([P, N_F8F, T], F8E4, name="ht8f")    # 0,1 all
            ht8h = ht_pool.tile([P, N_F8H, HALF], F8E4, name="ht8h")  # 2,3 lo-half

            # ---- stage 1: two column-pair passes (p=0: chunks 0/2, p=1: 1/3)
            for p in range(2):
                if p == 0:
                    cA, cB = cA0, cB0      # token halves: B11/B21 and B12/B22
                else:
                    cA, cB = chunk_tiles()
                    # chunk 1: WAR on cA clears when p=0 phase A is done
                    nc.sync.dma_start(out=cA[:], in_=xtp[:, 1, :, :])
                m2st = m2s_pool.tile([P, N_MT, TSUB], F16, tag="m2s")
                tt = tt_pool.tile([P, 5, HK, TSUB], F16, tag="tt")

                # T4 = B21 - B11 (chunk A only; emitted first on the vector
                # FIFO so it runs as soon as chunk A lands -- phase B's first
                # product consumes it)
                nc.vector.tensor_tensor(tt[:, 1, :, :], cA[:, HK:, :],
                                        cA[:, 0:HK, :], SUB)

                # phase A: M2 = (A21+A22) @ B11 for all mt (no DVE deps)
                for mt in range(N_MT):
                    if p == 0 and mt in a2_tiles:
                        at = a2_tiles[mt]
                    else:
                        at = load_a(mt, M2)
                    psm = ps_pool.tile([P, TSUB], F32, bufs=3, name="psm")
                    for kb in range(HK):
                        nc.tensor.matmul(psm[:], lhsT=at[:, kb, :],
                                         rhs=cA[:, kb, :],
                                         start=(kb == 0), stop=(kb == HK - 1))
                    nc.vector.tensor_copy(m2st[:, mt, :], psm[:])

                if p == 1:
                    # chunk 3 into cB (WAR: after p=0's last B22 matmul)
                    nc.sync.dma_start(out=cB[:], in_=xtp[:, 3, :, :])
                # remaining T sums (need chunk B; ready before phase B's
                # third product)
                # T1 = B11 + B22
                nc.vector.tensor_tensor(tt[:, 0, :, :], cA[:, 0:HK, :],
                                        cB[:, HK:, :], ADD)
                # T3 = B12 - B22
                nc.vector.tensor_tensor(tt[:, 2, :, :], cB[:, 0:HK, :],
                                        cB[:, HK:, :], SUB)
                # T6 = B11 + B12
                nc.vector.tensor_tensor(tt[:, 3, :, :], cA[:, 0:HK, :],
                                        cB[:, 0:HK, :], ADD)
                # T7 = B21 + B22
                nc.vector.tensor_tensor(tt[:, 4, :, :], cA[:, HK:, :],
                                        cB[:, HK:, :], ADD)

                rhs_by_m = {
                    0: tt[:, 0, :, :],      # M1: T1
                    2: tt[:, 2, :, :],      # M3: T3
                    3: tt[:, 1, :, :],      # M4: T4
                    4: cB[:, HK:, :],       # M5: B22 raw
                    5: tt[:, 3, :, :],      # M6: T6
                    6: tt[:, 4, :, :],      # M7: T7
                }

                # phase B: remaining 6 products per mt + recombine + swiglu
                for mt in range(N_MT):
                    mts = ms_pool.tile([P, 6, TSUB], F16, tag="ms")
                    for j, m in enumerate(PHASE_B):
                        at = load_a(mt, m)
                        psm = ps_pool.tile([P, TSUB], F32, bufs=3, name="psm")
                        rhs = rhs_by_m[m]
                        for kb in range(HK):
                            nc.tensor.matmul(psm[:], lhsT=at[:, kb, :],
                                             rhs=rhs[:, kb, :],
                                             start=(kb == 0),
                                             stop=(kb == HK - 1))
                        nc.vector.tensor_copy(mts[:, j, :], psm[:])
                    m1 = mts[:, 2, :]
                    m3 = mts[:, 3, :]
                    m4 = mts[:, 0, :]
                    m5 = mts[:, 1, :]
                    m6 = mts[:, 4, :]
                    m7 = mts[:, 5, :]
                    m2 = m2st[:, mt, :]
                    cc = cc_pool.tile([P, 6, TSUB], F32, tag="cc")
                    c11, c12, c21, c22, s0, s1 = (cc[:, i, :] for i in range(6))
                    # C11 = M1 + M4 - M5 + M7  (no in-place DVE ops)
                    nc.vector.tensor_tensor(s0, m1, m4, ADD)
                    nc.vector.tensor_tensor(s1, s0, m5, SUB)
                    nc.vector.tensor_tensor(c11, s1, m7, ADD)
                    # C21 = M2 + M4
                    nc.vector.tensor_tensor(c21, m2, m4, ADD)
                    # C12 = M3 + M5
                    nc.vector.tensor_tensor(c12, m3, m5, ADD)
                    # C22 = M1 - M2 + M3 + M6
                    nc.vector.tensor_tensor(s0, m1, m2, SUB)
                    nc.vector.tensor_tensor(s1, s0, m3, ADD)
                    nc.vector.tensor_tensor(c22, s1, m6, ADD)

                    lo = p * TSUB            # token cols within each half
                    # half 1 (tokens 0:1024): fp8 for mt<4, fp16 otherwise
                    tmp = tmp_pool.tile([P, TSUB], F32, tag="tmp")
                    nc.scalar.activation(tmp[:], c11, SILU)
                    if mt < 2:
                        bsc = tmp_pool.tile([P, TSUB], F32, tag="tmp")
                        nc.scalar.activation(bsc[:], c21, COPY, scale=H8_SCALE)
                        nc.vector.tensor_tensor(
                            ht8f[:, mt, lo:lo + TSUB], tmp[:], bsc[:],
                            mybir.AluOpType.mult)
                    elif mt < 4:
                        bsc = tmp_pool.tile([P, TSUB], F32, tag="tmp")
                        nc.scalar.activation(bsc[:], c21, COPY, scale=H8_SCALE)
                        nc.vector.tensor_tensor(
                            ht8h[:, mt - 2, lo:lo + TSUB], tmp[:], bsc[:],
                            mybir.AluOpType.mult)
                    else:
                        nc.vector.tensor_tensor(
                            ht[:, mt - 4, lo:lo + TSUB], tmp[:], c21,
                            mybir.AluOpType.mult)
                    # half 2 (tokens 1024:2048): fp8 only for mt<2
                    tmp2 = tmp_pool.tile([P, TSUB], F32, tag="tmp")
                    nc.scalar.activation(tmp2[:], c12, SILU)
                    if mt < 2:
                        bsc2 = tmp_pool.tile([P, TSUB], F32, tag="tmp")
                        nc.scalar.activation(bsc2[:], c22, COPY, scale=H8_SCALE)
                        nc.vector.tensor_tensor(
                            ht8f[:, mt, HALF + lo:HALF + lo + TSUB],
                            tmp2[:], bsc2[:], mybir.AluOpType.mult)
                    elif mt < 4:
                        nc.vector.tensor_tensor(
                            ht23[:, mt - 2, lo:lo + TSUB], tmp2[:], c22,
                            mybir.AluOpType.mult)
                    else:
                        nc.vector.tensor_tensor(
                            ht[:, mt - 4, HALF + lo:HALF + lo + TSUB],
                            tmp2[:], c22, mybir.AluOpType.mult)

            # ---- stage 2: OUT.T[d, t] = sum_h W2T[h, d] * HT[h, t]
            NTS = T // TSUB
            for idt in range(N_DT):
                w2i = w2_pool.tile([P, N_W2F, P], F16, tag="w2")
                nc.sync.dma_start(out=w2i[:], in_=w2p[:, idt, :, :])
                w28i = w28_pool.tile([P, N_F8F + N_F8H, P], F8E4, tag="w28")
                nc.sync.dma_start(out=w28i[:], in_=w28p[:, idt, :, :])
                ob = ob_pool.tile([P, T], F16)
                for its in range(NTS):
                    seg = slice(its * TSUB, (its + 1) * TSUB)
                    pso = ps_pool.tile([P, TSUB], F32, bufs=4, name="pso")
                    nc.tensor.matmul(pso[:], lhsT=w28i[:, 0:2, :],
                                     rhs=ht8f[:, :, seg], start=True,
                                     stop=False, perf_mode=DROW)
                    if its < 2:
                        nc.tensor.matmul(pso[:], lhsT=w28i[:, 2:4, :],
                                         rhs=ht8h[:, :, seg], start=False,
                                         stop=False, perf_mode=DROW)
                    else:
                        hseg = slice(its * TSUB - HALF, (its + 1) * TSUB - HALF)
                        for hb in range(N_F8H):
                            nc.tensor.matmul(pso[:], lhsT=w2i[:, hb, :],
                                             rhs=ht23[:, hb, hseg],
                                             start=False, stop=False)
                    for j in range(N_HB):
                        nc.tensor.matmul(pso[:], lhsT=w2i[:, N_F8H + j, :],
                                         rhs=ht[:, j, seg], start=False,
                                         stop=(j == N_HB - 1))
                    if idt < N_DT - 1 or its < NTS - 1:
                        nc.scalar.activation(ob[:, seg], pso[:], COPY,
                                             scale=1.0 / W2F_SCALE)
                        if idt == N_DT - 1:
                            nc.sync.dma_start(
                                out=outt[idt * P:(idt + 1) * P, seg],
                                in_=ob[:, seg])
                    else:
                        # final chunk: 2x256 with acts/DMAs on separate
                        # queues so descriptor programming overlaps
                        for h in range(2):
                            lo = its * TSUB + h * (TSUB // 2)
                            sl = slice(lo, lo + TSUB // 2)
                            psl = slice(h * (TSUB // 2), (h + 1) * (TSUB // 2))
                            nc.scalar.activation(ob[:, sl], pso[:, psl],
                                                 COPY, scale=1.0 / W2F_SCALE)
                            eng = nc.sync if h == 0 else nc.gpsimd
                            eng.dma_start(out=outt[idt * P:(idt + 1) * P, sl],
                                          in_=ob[:, sl])
                if idt < N_DT - 1:
                    nc.sync.dma_start(out=outt[idt * P:(idt + 1) * P, :],
                                      in_=ob[:])
    nc.compile()
    return nc


_CACHE: dict = {}


def _get_nc() -> bass.Bass:
    if "nc" not in _CACHE:
        _CACHE["nc"] = _build_program()
    return _CACHE["nc"]


def _pack_weights(w1, w2, w3):
    maps = []
    for e in range(N_EXPERTS):
        a11 = w1[e][:, :DIM // 2]
        a12 = w1[e][:, DIM // 2:]
        a21 = w3[e][:, :DIM // 2]
        a22 = w3[e][:, DIM // 2:]
        am = np.stack([a11 + a22, a21 + a22, a11, a22,
                       a11 + a12, a21 - a11, a12 - a22], 0)  # [7, 1408, 1024]
        apk = np.ascontiguousarray(
            am.reshape(7, N_MT, P, HK, P).transpose(4, 1, 0, 3, 2)
            .astype(NPF16))
        w2f = np.ascontiguousarray(
            (W2F_SCALE * w2[e][:, N_F8F * P:])
            .reshape(N_DT, P, N_W2F, P).transpose(3, 0, 2, 1).astype(NPF16))
        w28 = np.clip(w2[e][:, :(N_F8F + N_F8H) * P] * W28_SCALE, -240.0, 240.0)
        w28 = np.ascontiguousarray(
            w28.reshape(N_DT, P, N_F8F + N_F8H, P).transpose(3, 0, 2, 1)
            .astype(F8))
        maps.append({"ap": apk, "w2p": w2f, "w28p": w28})
    return maps


def kernel(x, w1, w2, w3, num_tokens_per_expert, _trace=False):
    x = np.ascontiguousarray(np.asarray(x, dtype=np.float32))
    w1 = np.ascontiguousarray(np.asarray(w1, dtype=np.float32))
    w2 = np.ascontiguousarray(np.asarray(w2, dtype=np.float32))
    w3 = np.ascontiguousarray(np.asarray(w3, dtype=np.float32))
    counts = np.asarray(num_tokens_per_expert, dtype=np.int64)

    cs = np.cumsum(counts)
    starts = np.minimum(np.concatenate([[0], cs[:-1]]), N_TOKENS)
    ends = np.minimum(cs, N_TOKENS)
    lens = np.maximum(ends - starts, 0)

    wmaps = _pack_weights(w1, w2, w3)
    out = np.zeros((N_TOKENS, DIM), np.float32)
    trace_info = []

    n_passes = max(1, int(np.max(np.ceil(lens / T))))
    for k in range(n_passes):
        in_maps = []
        for e in range(N_EXPERTS):
            s = int(starts[e]) + k * T
            xe = np.zeros((T, DIM), np.float32)
            avail = x[s:s + T]
            if avail.shape[0]:
                xe[:avail.shape[0]] = avail
            xtp = np.ascontiguousarray(
                xe.T.reshape(N_DN, P, T // TSUB, TSUB)
                .transpose(1, 2, 0, 3).astype(NPF16))
            in_maps.append({"xtp": xtp, **wmaps[e]})
        res = run_bass_kernel_spmd(
            _get_nc(), in_maps, list(range(N_CORES)), trace=_trace
        )
        if _trace:
            trace_info.append(res)
        for e in range(N_EXPERTS):
            s = int(starts[e]) + k * T
            cnt = min(int(ends[e]) - s, T)
            if cnt > 0:
                out[s:s + cnt] = res.results[e]["outt"].T[:cnt].astype(np.float32)

    if _trace:
        return out, trace_info
    return out


# revision 20
# speedup vs baseline: 1.0989x; 1.0109x over previous
"""Grouped-experts SwiGLU FFN (MoE) on 8 Trainium2 NeuronCores.

Expert-parallel: core e owns expert e's weights and its contiguous token
slice (tokens are pre-sorted by expert).  Per core, out.T = W2 @ h where
h = silu(W1 x) * (W3 x), x [2048 dim, 2048 tok].

Stage 1 runs one level of Strassen on the stacked [W1; W3] @ x product:
A = [W1; W3] is [2816, 2048] (M-halves of 1408 = 11 clean 128-row tiles),
K = 2048 and N = 2048 both split 1024.  7 M-products instead of 8
block-products cuts stage-1 PE streaming by 12.5% (1232 vs 1408 matmuls).
The A-side combinations are folded into host weight packing; the B-side
(x) sums are 5 DVE adds per token-column-pair; M-product drains and the
C recombination ride the otherwise-idle Vector engine under the PE
shadow.  silu(C_top) * C_bot then feeds stage 2 unchanged.

Precision: fp16 operands everywhere (same PE speed as bf16, 8x lower
noise), plus e4m3 fp8 DoubleRow (2 contraction blocks per matmul) for
stage-2 h-blocks 0,1 on all tokens and blocks 2,3 on tokens 0:1024.
Offline-simulated rel err 1.958e-2 vs the 2e-2 budget (HW measured
tracks the simulator within ~3e-4).  fp16 w2 is host-scaled x128 so its
PSUM partials match the (4h)(32w2) fp8 partials; drains scale by 1/128.

Head: the runtime preamble is ~7us; a few junk warm-up matmuls issued
first get the PE HAM clock-gate to 2.4 GHz before real operands land,
and the DMA descriptor order streams exactly what the first matmul
chain needs (A(m2) tiles + x chunk 0) at full bandwidth.
"""

import numpy as np
import ml_dtypes

import concourse.bass as bass
from concourse import bacc
import concourse.mybir as mybir
from concourse.tile import TileContext
from concourse.bass_utils import run_bass_kernel_spmd

N_TOKENS = 16384
DIM = 2048
HIDDEN = 1408
N_EXPERTS = 8
N_CORES = 8

P = 128
T = 2048                 # token capacity per core per pass
N_DN = DIM // P          # 16 k-blocks (full K)
HK = 8                   # k-blocks per Strassen half (1024/128)
N_MT = HIDDEN // P       # 11 output-row tiles per M-product
N_HT = N_MT              # 11 h blocks
N_F8F = 2                # h-blocks 0,1: fp8 DR for all tokens
N_F8H = 2                # h-blocks 2,3: fp8 DR for tokens 0:1024
N_HB = 7                 # h-blocks 4..10: fp16-only path
N_W2F = 9                # fp16 w2 blocks 2..10 (2,3 used for tokens 1024:2048)
N_DT = DIM // P          # 16 output-row tiles in stage 2
TSUB = 512
HALF = T // 2

F32 = mybir.dt.float32
F16 = mybir.dt.float16
F8E4 = mybir.dt.float8e4
SILU = mybir.ActivationFunctionType.Silu
COPY = mybir.ActivationFunctionType.Copy
DROW = mybir.MatmulPerfMode.DoubleRow
ADD = mybir.AluOpType.add
SUB = mybir.AluOpType.subtract
NPF16 = np.float16
F8 = ml_dtypes.float8_e4m3
H8_SCALE = 4.0           # ht8 = e4m3(4h)
W28_SCALE = 32.0         # fp8 w2 blocks x32 -> PSUM partial 128*out
W2F_SCALE = 128.0        # fp16 w2 x128 -> matches fp8 partial scale

# Strassen M-product index order within phase B (phase A runs m2 alone):
# m4 first (its T-sum needs only x chunk A), then m5 (raw), m1/m3/m6/m7.
M2 = 1                   # m-index (0-based) of M2 = (A21+A22) B11
PHASE_B = [3, 4, 0, 2, 5, 6]   # m4, m5, m1, m3, m6, m7
N_WU = 8                 # warm-up matmuls during the runtime preamble


def _build_program() -> bass.Bass:
    nc = bacc.Bacc()
    # A combinations: ap[p, mt, m, kb, c] = A_m[mt*128+c, kb*128+p]
    ap = nc.declare_dram_parameter("ap", [P, N_MT, 7, HK, P], F16, isOutput=False)
    xtp = nc.declare_dram_parameter(
        "xtp", [P, T // TSUB, N_DN, TSUB], F16, isOutput=False)
    w2p = nc.declare_dram_parameter("w2p", [P, N_DT, N_W2F, P], F16, isOutput=False)
    w28p = nc.declare_dram_parameter(
        "w28p", [P, N_DT, N_F8F + N_F8H, P], F8E4, isOutput=False)
    outt = nc.declare_dram_parameter("outt", [DIM, T], F16, isOutput=True)

    with TileContext(nc) as tc:
        with (
            tc.tile_pool(name="wu", bufs=1) as wu_pool,
            tc.tile_pool(name="xt", bufs=1) as xt_pool,
            tc.tile_pool(name="at", bufs=8) as at_pool,
            tc.tile_pool(name="tt", bufs=1) as tt_pool,
            tc.tile_pool(name="m2s", bufs=1) as m2s_pool,
            tc.tile_pool(name="ms", bufs=2) as ms_pool,
            tc.tile_pool(name="cc", bufs=1) as cc_pool,
            tc.tile_pool(name="ht", bufs=1) as ht_pool,
            tc.tile_pool(name="w2", bufs=2) as w2_pool,
            tc.tile_pool(name="w28", bufs=2) as w28_pool,
            tc.tile_pool(name="tmp", bufs=4) as tmp_pool,
            tc.tile_pool(name="ob", bufs=2) as ob_pool,
            tc.tile_pool(name="ps", bufs=1, space="PSUM") as ps_pool,
        ):
            # ---- PE warm-up: junk matmuls issued before any DMA lands so
            # the HAM clock-gate reaches 2.4 GHz during the ~7us preamble.
            # memset on gpsimd: its sequencer is ready ~2us before vector's.
            wu = wu_pool.tile([P, TSUB], F16)
            nc.gpsimd.memset(wu[:], 0.0)
            for _ in range(N_WU):
                psw = ps_pool.tile([P, TSUB], F32, bufs=4, name="psm")
                nc.tensor.matmul(psw[:], lhsT=wu[:, 0:P], rhs=wu[:],
                                 start=True, stop=True)

            # ---- bulk input tiles: chunk pair (cA, cB) per column pass;
            # p=1 reuses p=0's buffers (WAR clears once p=0 stops reading)
            def chunk_tiles():
                cA = xt_pool.tile([P, N_DN, TSUB], F16, bufs=1, tag="xta",
                                  name="xcA")
                cB = xt_pool.tile([P, N_DN, TSUB], F16, bufs=1, tag="xtb",
                                  name="xcB")
                return cA, cB

            def load_a(mt, m):
                at = at_pool.tile([P, HK, P], F16, tag="at")
                nc.sync.dma_start(out=at[:], in_=ap[:, mt, m, :, :])
                return at

            # Head-critical DMA order: A(mt0,m2), x chunk0 (fine-grained so
            # the first matmul chain starts ASAP), more A(m2), x chunk2.
            # Head-critical descriptors, finest first: the opening matmul
            # needs only A2[mt0,kb0] (32KB) + x[kb0] (128KB).
            cA0, cB0 = chunk_tiles()
            a2_tiles = {}
            a0 = at_pool.tile([P, HK, P], F16, tag="at")
            nc.sync.dma_start(out=a0[:, 0:1, :], in_=ap[:, 0, M2, 0:1, :])
            nc.sync.dma_start(out=cA0[:, 0:1, :], in_=xtp[:, 0, 0:1, :])
            nc.sync.dma_start(out=a0[:, 1:, :], in_=ap[:, 0, M2, 1:, :])
            a2_tiles[0] = a0
            nc.sync.dma_start(out=cA0[:, 1:4, :], in_=xtp[:, 0, 1:4, :])
            a2_tiles[1] = load_a(1, M2)
            nc.sync.dma_start(out=cA0[:, 4:8, :], in_=xtp[:, 0, 4:8, :])
            a2_tiles[2] = load_a(2, M2)
            a2_tiles[3] = load_a(3, M2)
            nc.sync.dma_start(out=cA0[:, HK:, :], in_=xtp[:, 0, HK:, :])
            a2_tiles[4] = load_a(4, M2)
            for q in range(4):
                nc.sync.dma_start(out=cB0[:, q * 4:(q + 1) * 4, :],
                                  in_=xtp[:, 2, q * 4:(q + 1) * 4, :])

            # ---- stage-1 outputs
            ht = ht_pool.tile([P, N_HB, T], F16)                 # blocks 4..10
            ht23 = ht_pool.tile([P, N_F8H, HALF], F16, name="ht23")  # 2,3 hi-half
            ht8f = ht_pool.tile([P, N_F8F, T], F8E4, name="ht8f")    # 0,1 all
            ht8h = ht_pool.tile([P, N_F8H, HALF], F8E4, name="ht8h")  # 2,3 lo-half

            # ---- stage 1: two column-pair passes (p=0: chunks 0/2, p=1: 1/3)
            for p in range(2):
                if p == 0:
                    cA, cB = cA0, cB0      # token halves: B11/B21 and B12/B22
                else:
                    cA, cB = chunk_tiles()
                    # chunk 1: WAR on cA clears when p=0 phase A is done
                    nc.sync.dma_start(out=cA[:], in_=xtp[:, 1, :, :])
                m2st = m2s_pool.tile([P, N_MT, TSUB], F16, tag="m2s")
                tt = tt_pool.tile([P, 5, HK, TSUB], F16, tag="tt")

                # T4 = B21 - B11 (chunk A only; emitted first on the vector
                # FIFO so it runs as soon as chunk A lands -- phase B's first
                # product consumes it)
                nc.vector.tensor_tensor(tt[:, 1, :, :], cA[:, HK:, :],
                                        cA[:, 0:HK, :], SUB)

                # phase A: M2 = (A21+A22) @ B11 for all mt (no DVE deps)
                for mt in range(N_MT):
                    if p == 0 and mt in a2_tiles:
                        at = a2_tiles[mt]
                    else:
                        at = load_a(mt, M2)
                    psm = ps_pool.tile([P, TSUB], F32, bufs=4, name="psm")
                    for kb in range(HK):
                        nc.tensor.matmul(psm[:], lhsT=at[:, kb, :],
                                         rhs=cA[:, kb, :],
                                         start=(kb == 0), stop=(kb == HK - 1))
                    # drain on the (idle) scalar engine -- keeps DVE free
                    nc.scalar.activation(m2st[:, mt, :], psm[:], COPY)

                if p == 1:
                    # chunk 3 into cB (WAR: after p=0's last B22 matmul)
                    nc.sync.dma_start(out=cB[:], in_=xtp[:, 3, :, :])
                # remaining T sums (need chunk B; ready before phase B's
                # third product)
                # T1 = B11 + B22
                nc.vector.tensor_tensor(tt[:, 0, :, :], cA[:, 0:HK, :],
                                        cB[:, HK:, :], ADD)
                # T3 = B12 - B22
                nc.vector.tensor_tensor(tt[:, 2, :, :], cB[:, 0:HK, :],
                                        cB[:, HK:, :], SUB)
                # T6 = B11 + B12
                nc.vector.tensor_tensor(tt[:, 3, :, :], cA[:, 0:HK, :],
                                        cB[:, 0:HK, :], ADD)
                # T7 = B21 + B22
                nc.vector.tensor_tensor(tt[:, 4, :, :], cA[:, HK:, :],
                                        cB[:, HK:, :], ADD)

                rhs_by_m = {
                    0: tt[:, 0, :, :],      # M1: T1
                    2: tt[:, 2, :, :],      # M3: T3
                    3: tt[:, 1, :, :],      # M4: T4
                    4: cB[:, HK:, :],       # M5: B22 raw
                    5: tt[:, 3, :, :],      # M6: T6
                    6: tt[:, 4, :, :],      # M7: T7
                }

                # phase B: remaining 6 products per mt + recombine + swiglu.
                # M4/M5/M1/M3 drain to SBUF via scalar; M6/M7 stay in PSUM
                # and are consumed directly by the recombination adds.
                for mt in range(N_MT):
                    mts = ms_pool.tile([P, 4, TSUB], F16, tag="ms")
                    ps_keep = {}
                    for j, m in enumerate(PHASE_B):
                        at = load_a(mt, m)
                        psm = ps_pool.tile([P, TSUB], F32, bufs=4, name="psm")
                        rhs = rhs_by_m[m]
                        for kb in range(HK):
                            nc.tensor.matmul(psm[:], lhsT=at[:, kb, :],
                                             rhs=rhs[:, kb, :],
                                             start=(kb == 0),
                                             stop=(kb == HK - 1))
                        if j < 4:
                            nc.scalar.activation(mts[:, j, :], psm[:], COPY)
                        else:
                            ps_keep[m] = psm
                    m1 = mts[:, 2, :]
                    m3 = mts[:, 3, :]
                    m4 = mts[:, 0, :]
                    m5 = mts[:, 1, :]
                    m6 = ps_keep[5][:]
                    m7 = ps_keep[6][:]
                    m2 = m2st[:, mt, :]
                    cc = cc_pool.tile([P, 6, TSUB], F32, tag="cc")
                    c11, c12, c21, c22, s0, s1 = (cc[:, i, :] for i in range(6))
                    # C11 = M1 + M4 - M5 + M7  (no in-place DVE ops)
                    nc.vector.tensor_tensor(s0, m1, m4, ADD)
                    nc.vector.tensor_tensor(s1, s0, m5, SUB)
                    nc.vector.tensor_tensor(c11, s1, m7, ADD)
                    # C21 = M2 + M4
                    nc.vector.tensor_tensor(c21, m2, m4, ADD)
                    # C12 = M3 + M5
                    nc.vector.tensor_tensor(c12, m3, m5, ADD)
                    # C22 = M1 - M2 + M3 + M6
                    nc.vector.tensor_tensor(s0, m1, m2, SUB)
                    nc.vector.tensor_tensor(s1, s0, m3, ADD)
                    nc.vector.tensor_tensor(c22, s1, m6, ADD)

                    lo = p * TSUB            # token cols within each half
                    # half 1 (tokens 0:1024): fp8 for mt<4, fp16 otherwise
                    tmp = tmp_pool.tile([P, TSUB], F32, tag="tmp")
                    nc.scalar.activation(tmp[:], c11, SILU)
                    if mt < 2:
                        bsc = tmp_pool.tile([P, TSUB], F32, tag="tmp")
                        nc.scalar.activation(bsc[:], c21, COPY, scale=H8_SCALE)
                        nc.vector.tensor_tensor(
                            ht8f[:, mt, lo:lo + TSUB], tmp[:], bsc[:],
                            mybir.AluOpType.mult)
                    elif mt < 4:
                        bsc = tmp_pool.tile([P, TSUB], F32, tag="tmp")
                        nc.scalar.activation(bsc[:], c21, COPY, scale=H8_SCALE)
                        nc.vector.tensor_tensor(
                            ht8h[:, mt - 2, lo:lo + TSUB], tmp[:], bsc[:],
                            mybir.AluOpType.mult)
                    else:
                        nc.vector.tensor_tensor(
                            ht[:, mt - 4, lo:lo + TSUB], tmp[:], c21,
                            mybir.AluOpType.mult)
                    # half 2 (tokens 1024:2048): fp8 only for mt<2
                    tmp2 = tmp_pool.tile([P, TSUB], F32, tag="tmp")
                    nc.scalar.activation(tmp2[:], c12, SILU)
                    if mt < 2:
                        bsc2 = tmp_pool.tile([P, TSUB], F32, tag="tmp")
                        nc.scalar.activation(bsc2[:], c22, COPY, scale=H8_SCALE)
                        nc.vector.tensor_tensor(
                            ht8f[:, mt, HALF + lo:HALF + lo + TSUB],
                            tmp2[:], bsc2[:], mybir.AluOpType.mult)
                    elif mt < 4:
                        nc.vector.tensor_tensor(
                            ht23[:, mt - 2, lo:lo + TSUB], tmp2[:], c22,
                            mybir.AluOpType.mult)
                    else:
                        nc.vector.tensor_tensor(
                            ht[:, mt - 4, HALF + lo:HALF + lo + TSUB],
                            tmp2[:], c22, mybir.AluOpType.mult)

            # ---- stage 2: OUT.T[d, t] = sum_h W2T[h, d] * HT[h, t]
            NTS = T // TSUB
            for idt in range(N_DT):
                w2i = w2_pool.tile([P, N_W2F, P], F16, tag="w2")
                nc.sync.dma_start(out=w2i[:], in_=w2p[:, idt, :, :])
                w28i = w28_pool.tile([P, N_F8F + N_F8H, P], F8E4, tag="w28")
                nc.sync.dma_start(out=w28i[:], in_=w28p[:, idt, :, :])
                ob = ob_pool.tile([P, T], F16)
                for its in range(NTS):
                    seg = slice(its * TSUB, (its + 1) * TSUB)
                    pso = ps_pool.tile([P, TSUB], F32, bufs=4, name="pso")
                    nc.tensor.matmul(pso[:], lhsT=w28i[:, 0:2, :],
                                     rhs=ht8f[:, :, seg], start=True,
                                     stop=False, perf_mode=DROW)
                    if its < 2:
                        nc.tensor.matmul(pso[:], lhsT=w28i[:, 2:4, :],
                                         rhs=ht8h[:, :, seg], start=False,
                                         stop=False, perf_mode=DROW)
                    else:
                        hseg = slice(its * TSUB - HALF, (its + 1) * TSUB - HALF)
                        for hb in range(N_F8H):
                            nc.tensor.matmul(pso[:], lhsT=w2i[:, hb, :],
                                             rhs=ht23[:, hb, hseg],
                                             start=False, stop=False)
                    for j in range(N_HB):
                        nc.tensor.matmul(pso[:], lhsT=w2i[:, N_F8H + j, :],
                                         rhs=ht[:, j, seg], start=False,
                                         stop=(j == N_HB - 1))
                    if idt < N_DT - 1 or its < NTS - 1:
                        nc.scalar.activation(ob[:, seg], pso[:], COPY,
                                             scale=1.0 / W2F_SCALE)
                        if idt == N_DT - 1:
                            nc.sync.dma_start(
                                out=outt[idt * P:(idt + 1) * P, seg],
                                in_=ob[:, seg])
                    else:
                        # final chunk: 2x256 with acts/DMAs on separate
                        # queues so descriptor programming overlaps
                        for h in range(2):
                            lo = its * TSUB + h * (TSUB // 2)
                            sl = slice(lo, lo + TSUB // 2)
                            psl = slice(h * (TSUB // 2), (h + 1) * (TSUB // 2))
                            nc.scalar.activation(ob[:, sl], pso[:, psl],
                                                 COPY, scale=1.0 / W2F_SCALE)
                            eng = nc.sync if h == 0 else nc.gpsimd
                            eng.dma_start(out=outt[idt * P:(idt + 1) * P, sl],
                                          in_=ob[:, sl])
                if idt < N_DT - 1:
                    nc.sync.dma_start(out=outt[idt * P:(idt + 1) * P, :],
                                      in_=ob[:])
    nc.compile()
    return nc


_CACHE: dict = {}


def _get_nc() -> bass.Bass:
    if "nc" not in _CACHE:
        _CACHE["nc"] = _build_program()
    return _CACHE["nc"]


def _pack_weights(w1, w2, w3):
    maps = []
    for e in range(N_EXPERTS):
        a11 = w1[e][:, :DIM // 2]
        a12 = w1[e][:, DIM // 2:]
        a21 = w3[e][:, :DIM // 2]
        a22 = w3[e][:, DIM // 2:]
        am = np.stack([a11 + a22, a21 + a22, a11, a22,
                       a11 + a12, a21 - a11, a12 - a22], 0)  # [7, 1408, 1024]
        apk = np.ascontiguousarray(
            am.reshape(7, N_MT, P, HK, P).transpose(4, 1, 0, 3, 2)
            .astype(NPF16))
        w2f = np.ascontiguousarray(
            (W2F_SCALE * w2[e][:, N_F8F * P:])
            .reshape(N_DT, P, N_W2F, P).transpose(3, 0, 2, 1).astype(NPF16))
        w28 = np.clip(w2[e][:, :(N_F8F + N_F8H) * P] * W28_SCALE, -240.0, 240.0)
        w28 = np.ascontiguousarray(
            w28.reshape(N_DT, P, N_F8F + N_F8H, P).transpose(3, 0, 2, 1)
            .astype(F8))
        maps.append({"ap": apk, "w2p": w2f, "w28p": w28})
    return maps


def kernel(x, w1, w2, w3, num_tokens_per_expert, _trace=False):
    x = np.ascontiguousarray(np.asarray(x, dtype=np.float32))
    w1 = np.ascontiguousarray(np.asarray(w1, dtype=np.float32))
    w2 = np.ascontiguousarray(np.asarray(w2, dtype=np.float32))
    w3 = np.ascontiguousarray(np.asarray(w3, dtype=np.float32))
    counts = np.asarray(num_tokens_per_expert, dtype=np.int64)

    cs = np.cumsum(counts)
    starts = np.minimum(np.concatenate([[0], cs[:-1]]), N_TOKENS)
    ends = np.minimum(cs, N_TOKENS)
    lens = np.maximum(ends - starts, 0)

    wmaps = _pack_weights(w1, w2, w3)
    out = np.zeros((N_TOKENS, DIM), np.float32)
    trace_info = []

    n_passes = max(1, int(np.max(np.ceil(lens / T))))
    for k in range(n_passes):
        in_maps = []
        for e in range(N_EXPERTS):
            s = int(starts[e]) + k * T
            xe = np.zeros((T, DIM), np.float32)
            avail = x[s:s + T]
            if avail.shape[0]:
                xe[:avail.shape[0]] = avail
            xtp = np.ascontiguousarray(
                xe.T.reshape(N_DN, P, T // TSUB, TSUB)
                .transpose(1, 2, 0, 3).astype(NPF16))
            in_maps.append({"xtp": xtp, **wmaps[e]})
        res = run_bass_kernel_spmd(
            _get_nc(), in_maps, list(range(N_CORES)), trace=_trace
        )
        if _trace:
            trace_info.append(res)
        for e in range(N_EXPERTS):
            s = int(starts[e]) + k * T
            cnt = min(int(ends[e]) - s, T)
            if cnt > 0:
                out[s:s + cnt] = res.results[e]["outt"].T[:cnt].astype(np.float32)

    if _trace:
        return out, trace_info
    return out


# revision 26
# speedup vs baseline: 1.1144x; 1.0141x over previous
"""Grouped-experts SwiGLU FFN (MoE) on 8 Trainium2 NeuronCores.

Expert-parallel: core e owns expert e's weights and its contiguous token
slice (tokens are pre-sorted by expert).  Per core, out.T = W2 @ h where
h = silu(W1 x) * (W3 x), x [2048 dim, 2048 tok].

Stage 1 runs one level of Strassen on the stacked [W1; W3] @ x product:
A = [W1; W3] is [2816, 2048] (M-halves of 1408 = 11 clean 128-row tiles),
K = 2048 and N = 2048 both split 1024.  7 M-products instead of 8
block-products cuts stage-1 PE streaming by 12.5% (1232 vs 1408 matmuls).
The A-side combinations are folded into host weight packing; the B-side
(x) sums are 5 DVE adds per token-column-pair; M-product drains and the
C recombination ride the otherwise-idle Vector engine under the PE
shadow.  silu(C_top) * C_bot then feeds stage 2 unchanged.

Precision: fp16 operands everywhere (same PE speed as bf16, 8x lower
noise), plus e4m3 fp8 DoubleRow (2 contraction blocks per matmul) for
stage-2 h-blocks 0,1 on all tokens and blocks 2,3 on tokens 0:1024.
Offline-simulated rel err 1.958e-2 vs the 2e-2 budget (HW measured
tracks the simulator within ~3e-4).  fp16 w2 is host-scaled x128 so its
PSUM partials match the (4h)(32w2) fp8 partials; drains scale by 1/128.

Head: the runtime preamble is ~7us; a few junk warm-up matmuls issued
first get the PE HAM clock-gate to 2.4 GHz before real operands land,
and the DMA descriptor order streams exactly what the first matmul
chain needs (A(m2) tiles + x chunk 0) at full bandwidth.
"""

import numpy as np
import ml_dtypes

import concourse.bass as bass
from concourse import bacc
import concourse.mybir as mybir
from concourse.tile import TileContext
from concourse.bass_utils import run_bass_kernel_spmd

N_TOKENS = 16384
DIM = 2048
HIDDEN = 1408
N_EXPERTS = 8
N_CORES = 8

P = 128
T = 2048                 # token capacity per core per pass
N_DN = DIM // P          # 16 k-blocks (full K)
HK = 8                   # k-blocks per Strassen half (1024/128)
N_MT = HIDDEN // P       # 11 output-row tiles per M-product
N_HT = N_MT              # 11 h blocks
N_F8F = 2                # h-blocks 0,1: fp8 DR for all tokens
N_F8H = 2                # h-blocks 2,3: fp8 DR for tokens 0:1024
N_HB = 7                 # h-blocks 4..10: fp16-only path
N_W2F = 9                # fp16 w2 blocks 2..10 (2,3 used for tokens 1024:2048)
N_DT = DIM // P          # 16 output-row tiles in stage 2
TSUB = 512
HALF = T // 2

F32 = mybir.dt.float32
F16 = mybir.dt.float16
F8E4 = mybir.dt.float8e4
SILU = mybir.ActivationFunctionType.Silu
COPY = mybir.ActivationFunctionType.Copy
DROW = mybir.MatmulPerfMode.DoubleRow
ADD = mybir.AluOpType.add
SUB = mybir.AluOpType.subtract
NPF16 = np.float16
F8 = ml_dtypes.float8_e4m3
H8_SCALE = 4.0           # ht8 = e4m3(4h)
W28_SCALE = 32.0         # fp8 w2 blocks x32 -> PSUM partial 128*out
W2F_SCALE = 128.0        # fp16 w2 x128 -> matches fp8 partial scale

# Strassen M-product index order within phase B (phase A runs m2 alone):
# m4 first (its T-sum needs only x chunk A), then m5 (raw), m1/m3/m6/m7.
M2 = 1                   # m-index (0-based) of M2 = (A21+A22) B11
PHASE_B = [3, 4, 0, 2, 5, 6]   # m4, m5, m1, m3, m6, m7
N_WU = 8                 # warm-up matmuls during the runtime preamble


def _build_program() -> bass.Bass:
    nc = bacc.Bacc()
    # A combinations: ap[p, mt, m, kb, c] = A_m[mt*128+c, kb*128+p]
    ap = nc.declare_dram_parameter("ap", [P, N_MT, 7, HK, P], F16, isOutput=False)
    xtp = nc.declare_dram_parameter(
        "xtp", [P, T // TSUB, N_DN, TSUB], F16, isOutput=False)
    w2p = nc.declare_dram_parameter("w2p", [P, N_DT, N_W2F, P], F16, isOutput=False)
    w28p = nc.declare_dram_parameter(
        "w28p", [P, N_DT, N_F8F + N_F8H, P], F8E4, isOutput=False)
    outt = nc.declare_dram_parameter("outt", [DIM, T], F16, isOutput=True)

    with TileContext(nc) as tc:
        with (
            tc.tile_pool(name="wu", bufs=1) as wu_pool,
            tc.tile_pool(name="xt", bufs=1) as xt_pool,
            tc.tile_pool(name="at", bufs=8) as at_pool,
            tc.tile_pool(name="tt", bufs=1) as tt_pool,
            tc.tile_pool(name="m2s", bufs=1) as m2s_pool,
            tc.tile_pool(name="ms", bufs=2) as ms_pool,
            tc.tile_pool(name="cc", bufs=1) as cc_pool,
            tc.tile_pool(name="ht", bufs=1) as ht_pool,
            tc.tile_pool(name="w2", bufs=2) as w2_pool,
            tc.tile_pool(name="w28", bufs=2) as w28_pool,
            tc.tile_pool(name="tmp", bufs=4) as tmp_pool,
            tc.tile_pool(name="ob", bufs=2) as ob_pool,
            tc.tile_pool(name="ps", bufs=1, space="PSUM") as ps_pool,
        ):
            # ---- PE warm-up: junk matmuls issued before any DMA lands so
            # the HAM clock-gate reaches 2.4 GHz during the ~7us preamble.
            # memset on gpsimd: its sequencer is ready ~2us before vector's.
            wu = wu_pool.tile([P, TSUB], F16)
            nc.gpsimd.memset(wu[:], 0.0)
            for _ in range(N_WU):
                psw = ps_pool.tile([P, TSUB], F32, bufs=4, name="psm")
                nc.tensor.matmul(psw[:], lhsT=wu[:, 0:P], rhs=wu[:],
                                 start=True, stop=True)

            # ---- bulk input tiles: chunk pair (cA, cB) per column pass;
            # p=1 reuses p=0's buffers (WAR clears once p=0 stops reading)
            # cA is split into three tiles: dependency tracking is per-tile,
            # so the opening matmul chain only waits on the 512KB it needs.
            def chunk_tiles():
                cAlo1 = xt_pool.tile([P, HK // 2, TSUB], F16, bufs=1,
                                     tag="xta1", name="xcAlo1")
                cAlo2 = xt_pool.tile([P, HK // 2, TSUB], F16, bufs=1,
                                     tag="xta2", name="xcAlo2")
                cAhi = xt_pool.tile([P, HK, TSUB], F16, bufs=1,
                                    tag="xth", name="xcAhi")
                cB = xt_pool.tile([P, N_DN, TSUB], F16, bufs=1, tag="xtb",
                                  name="xcB")
                return cAlo1, cAlo2, cAhi, cB

            def load_a(mt, m):
                at = at_pool.tile([P, HK, P], F16, tag="at")
                nc.sync.dma_start(out=at[:], in_=ap[:, mt, m, :, :])
                return at

            # Head-critical DMA order: A(mt0,m2), x chunk0 (fine-grained so
            # the first matmul chain starts ASAP), more A(m2), x chunk2.
            # Head-critical descriptors only: A2(mt0) + B11 of chunk 0 feed
            # the opening matmul chains.  Everything else (cAhi, chunk 2)
            # is emitted after phase A's A-tile loads so it cannot steal
            # bandwidth from them.
            cAlo1_0, cAlo2_0, cAhi0, cB0 = chunk_tiles()
            a2_tiles = {}
            a2_tiles[0] = load_a(0, M2)
            nc.sync.dma_start(out=cAlo1_0[:], in_=xtp[:, 0, 0:HK // 2, :])
            a2_tiles[1] = load_a(1, M2)
            nc.sync.dma_start(out=cAlo2_0[:], in_=xtp[:, 0, HK // 2:HK, :])
            a2_tiles[2] = load_a(2, M2)
            a2_tiles[3] = load_a(3, M2)
            a2_tiles[4] = load_a(4, M2)

            # ---- stage-1 outputs
            ht = ht_pool.tile([P, N_HB, T], F16)                 # blocks 4..10
            ht23 = ht_pool.tile([P, N_F8H, HALF], F16, name="ht23")  # 2,3 hi-half
            ht8f = ht_pool.tile([P, N_F8F, T], F8E4, name="ht8f")    # 0,1 all
            ht8h = ht_pool.tile([P, N_F8H, HALF], F8E4, name="ht8h")  # 2,3 lo-half

            # ---- stage 1: two column-pair passes (p=0: chunks 0/2, p=1: 1/3)
            for p in range(2):
                if p == 0:
                    cAlo1, cAlo2, cAhi, cB = cAlo1_0, cAlo2_0, cAhi0, cB0
                else:
                    cAlo1, cAlo2, cAhi, cB = chunk_tiles()
                    nc.sync.dma_start(out=cAlo1[:],
                                      in_=xtp[:, 1, 0:HK // 2, :])
                    nc.sync.dma_start(out=cAlo2[:],
                                      in_=xtp[:, 1, HK // 2:HK, :])
                    nc.sync.dma_start(out=cAhi[:], in_=xtp[:, 1, HK:, :])
                m2st = m2s_pool.tile([P, N_MT, TSUB], F16, tag="m2s")
                tt = tt_pool.tile([P, 5, HK, TSUB], F16, tag="tt")

                def calo(kb):
                    return (cAlo1[:, kb, :] if kb < HK // 2
                            else cAlo2[:, kb - HK // 2, :])

                # phase A: M2 = (A21+A22) @ B11 for all mt (no DVE deps)
                for mt in range(N_MT):
                    if p == 0 and mt in a2_tiles:
                        at = a2_tiles[mt]
                    else:
                        at = load_a(mt, M2)
                    psm = ps_pool.tile([P, TSUB], F32, bufs=4, name="psm")
                    for kb in range(HK):
                        nc.tensor.matmul(psm[:], lhsT=at[:, kb, :],
                                         rhs=calo(kb),
                                         start=(kb == 0), stop=(kb == HK - 1))
                    # drain on the (idle) scalar engine -- keeps DVE free
                    nc.scalar.activation(m2st[:, mt, :], psm[:], COPY)

                # bulk loads for this pass, behind the phase-A A-tiles
                if p == 0:
                    nc.sync.dma_start(out=cAhi[:], in_=xtp[:, 0, HK:, :])
                    for h in range(2):
                        nc.sync.dma_start(out=cB[:, h * HK:(h + 1) * HK, :],
                                          in_=xtp[:, 2, h * HK:(h + 1) * HK, :])
                else:
                    # chunk 3 into cB (WAR: after p=0's last B22 matmul)
                    nc.sync.dma_start(out=cB[:], in_=xtp[:, 3, :, :])
                # T sums (vector FIFO is otherwise idle during phase A, so
                # these run as soon as their chunks land; phase B's first
                # product needs T4)
                # T4 = B21 - B11
                nc.vector.tensor_tensor(tt[:, 1, 0:HK // 2, :],
                                        cAhi[:, 0:HK // 2, :], cAlo1[:], SUB)
                nc.vector.tensor_tensor(tt[:, 1, HK // 2:, :],
                                        cAhi[:, HK // 2:, :], cAlo2[:], SUB)
                # T1 = B11 + B22
                nc.vector.tensor_tensor(tt[:, 0, 0:HK // 2, :], cAlo1[:],
                                        cB[:, HK:HK + HK // 2, :], ADD)
                nc.vector.tensor_tensor(tt[:, 0, HK // 2:, :], cAlo2[:],
                                        cB[:, HK + HK // 2:, :], ADD)
                # T3 = B12 - B22
                nc.vector.tensor_tensor(tt[:, 2, :, :], cB[:, 0:HK, :],
                                        cB[:, HK:, :], SUB)
                # T6 = B11 + B12
                nc.vector.tensor_tensor(tt[:, 3, 0:HK // 2, :], cAlo1[:],
                                        cB[:, 0:HK // 2, :], ADD)
                nc.vector.tensor_tensor(tt[:, 3, HK // 2:, :], cAlo2[:],
                                        cB[:, HK // 2:HK, :], ADD)
                # T7 = B21 + B22
                nc.vector.tensor_tensor(tt[:, 4, :, :], cAhi[:],
                                        cB[:, HK:, :], ADD)

                rhs_by_m = {
                    0: tt[:, 0, :, :],      # M1: T1
                    2: tt[:, 2, :, :],      # M3: T3
                    3: tt[:, 1, :, :],      # M4: T4
                    4: cB[:, HK:, :],       # M5: B22 raw
                    5: tt[:, 3, :, :],      # M6: T6
                    6: tt[:, 4, :, :],      # M7: T7
                }

                # phase B: remaining 6 products per mt + recombine + swiglu.
                # M4/M5/M1/M3 drain to SBUF via scalar; M6/M7 stay in PSUM
                # and are consumed directly by the recombination adds.
                for mt in range(N_MT):
                    mts = ms_pool.tile([P, 4, TSUB], F16, tag="ms")
                    ps_keep = {}
                    for j, m in enumerate(PHASE_B):
                        at = load_a(mt, m)
                        psm = ps_pool.tile([P, TSUB], F32, bufs=4, name="psm")
                        rhs = rhs_by_m[m]
                        for kb in range(HK):
                            nc.tensor.matmul(psm[:], lhsT=at[:, kb, :],
                                             rhs=rhs[:, kb, :],
                                             start=(kb == 0),
                                             stop=(kb == HK - 1))
                        if j < 4:
                            nc.scalar.activation(mts[:, j, :], psm[:], COPY)
                        else:
                            ps_keep[m] = psm
                    m1 = mts[:, 2, :]
                    m3 = mts[:, 3, :]
                    m4 = mts[:, 0, :]
                    m5 = mts[:, 1, :]
                    m6 = ps_keep[5][:]
                    m7 = ps_keep[6][:]
                    m2 = m2st[:, mt, :]
                    cc = cc_pool.tile([P, 6, TSUB], F32, tag="cc")
                    c11, c12, c21, c22, s0, s1 = (cc[:, i, :] for i in range(6))
                    # C11 = M1 + M4 - M5 + M7  (no in-place DVE ops)
                    nc.vector.tensor_tensor(s0, m1, m4, ADD)
                    nc.vector.tensor_tensor(s1, s0, m5, SUB)
                    nc.vector.tensor_tensor(c11, s1, m7, ADD)
                    # C21 = M2 + M4
                    nc.vector.tensor_tensor(c21, m2, m4, ADD)
                    # C12 = M3 + M5
                    nc.vector.tensor_tensor(c12, m3, m5, ADD)
                    # C22 = M1 - M2 + M3 + M6
                    nc.vector.tensor_tensor(s0, m1, m2, SUB)
                    nc.vector.tensor_tensor(s1, s0, m3, ADD)
                    nc.vector.tensor_tensor(c22, s1, m6, ADD)

                    lo = p * TSUB            # token cols within each half
                    # half 1 (tokens 0:1024): fp8 for mt<4, fp16 otherwise
                    tmp = tmp_pool.tile([P, TSUB], F32, tag="tmp")
                    nc.scalar.activation(tmp[:], c11, SILU)
                    if mt < 2:
                        bsc = tmp_pool.tile([P, TSUB], F32, tag="tmp")
                        nc.scalar.activation(bsc[:], c21, COPY, scale=H8_SCALE)
                        nc.vector.tensor_tensor(
                            ht8f[:, mt, lo:lo + TSUB], tmp[:], bsc[:],
                            mybir.AluOpType.mult)
                    elif mt < 4:
                        bsc = tmp_pool.tile([P, TSUB], F32, tag="tmp")
                        nc.scalar.activation(bsc[:], c21, COPY, scale=H8_SCALE)
                        nc.vector.tensor_tensor(
                            ht8h[:, mt - 2, lo:lo + TSUB], tmp[:], bsc[:],
                            mybir.AluOpType.mult)
                    else:
                        nc.vector.tensor_tensor(
                            ht[:, mt - 4, lo:lo + TSUB], tmp[:], c21,
                            mybir.AluOpType.mult)
                    # half 2 (tokens 1024:2048): fp8 only for mt<2
                    tmp2 = tmp_pool.tile([P, TSUB], F32, tag="tmp")
                    nc.scalar.activation(tmp2[:], c12, SILU)
                    if mt < 2:
                        bsc2 = tmp_pool.tile([P, TSUB], F32, tag="tmp")
                        nc.scalar.activation(bsc2[:], c22, COPY, scale=H8_SCALE)
                        nc.vector.tensor_tensor(
                            ht8f[:, mt, HALF + lo:HALF + lo + TSUB],
                            tmp2[:], bsc2[:], mybir.AluOpType.mult)
                    elif mt < 4:
                        nc.vector.tensor_tensor(
                            ht23[:, mt - 2, lo:lo + TSUB], tmp2[:], c22,
                            mybir.AluOpType.mult)
                    else:
                        nc.vector.tensor_tensor(
                            ht[:, mt - 4, HALF + lo:HALF + lo + TSUB],
                            tmp2[:], c22, mybir.AluOpType.mult)

            # ---- stage 2: OUT.T[d, t] = sum_h W2T[h, d] * HT[h, t]
            NTS = T // TSUB
            for idt in range(N_DT):
                w2i = w2_pool.tile([P, N_W2F, P], F16, tag="w2")
                nc.sync.dma_start(out=w2i[:], in_=w2p[:, idt, :, :])
                w28i = w28_pool.tile([P, N_F8F + N_F8H, P], F8E4, tag="w28")
                nc.sync.dma_start(out=w28i[:], in_=w28p[:, idt, :, :])
                ob = ob_pool.tile([P, T], F16)
                for its in range(NTS):
                    seg = slice(its * TSUB, (its + 1) * TSUB)
                    pso = ps_pool.tile([P, TSUB], F32, bufs=4, name="pso")
                    nc.tensor.matmul(pso[:], lhsT=w28i[:, 0:2, :],
                                     rhs=ht8f[:, :, seg], start=True,
                                     stop=False, perf_mode=DROW)
                    if its < 2:
                        nc.tensor.matmul(pso[:], lhsT=w28i[:, 2:4, :],
                                         rhs=ht8h[:, :, seg], start=False,
                                         stop=False, perf_mode=DROW)
                    else:
                        hseg = slice(its * TSUB - HALF, (its + 1) * TSUB - HALF)
                        for hb in range(N_F8H):
                            nc.tensor.matmul(pso[:], lhsT=w2i[:, hb, :],
                                             rhs=ht23[:, hb, hseg],
                                             start=False, stop=False)
                    for j in range(N_HB):
                        nc.tensor.matmul(pso[:], lhsT=w2i[:, N_F8H + j, :],
                                         rhs=ht[:, j, seg], start=False,
                                         stop=(j == N_HB - 1))
                    if idt < N_DT - 1 or its < NTS - 1:
                        nc.scalar.activation(ob[:, seg], pso[:], COPY,
                                             scale=1.0 / W2F_SCALE)
                        if idt == N_DT - 1:
                            nc.sync.dma_start(
                                out=outt[idt * P:(idt + 1) * P, seg],
                                in_=ob[:, seg])
                    else:
                        # final chunk: 2x256, drained via scalar+vector in
                        # parallel, DMAs on separate queues so descriptor
                        # programming overlaps
                        for h in range(2):
                            lo = its * TSUB + h * (TSUB // 2)
                            sl = slice(lo, lo + TSUB // 2)
                            psl = slice(h * (TSUB // 2), (h + 1) * (TSUB // 2))
                            if h == 0:
                                nc.scalar.activation(ob[:, sl], pso[:, psl],
                                                     COPY,
                                                     scale=1.0 / W2F_SCALE)
                            else:
                                nc.vector.tensor_scalar_mul(
                                    ob[:, sl], pso[:, psl], 1.0 / W2F_SCALE)
                            eng = nc.sync if h == 0 else nc.gpsimd
                            eng.dma_start(out=outt[idt * P:(idt + 1) * P, sl],
                                          in_=ob[:, sl])
                if idt < N_DT - 1:
                    nc.sync.dma_start(out=outt[idt * P:(idt + 1) * P, :],
                                      in_=ob[:])
    nc.compile()
    return nc


_CACHE: dict = {}


def _get_nc() -> bass.Bass:
    if "nc" not in _CACHE:
        _CACHE["nc"] = _build_program()
    return _CACHE["nc"]


def _pack_weights(w1, w2, w3):
    maps = []
    for e in range(N_EXPERTS):
        a11 = w1[e][:, :DIM // 2]
        a12 = w1[e][:, DIM // 2:]
        a21 = w3[e][:, :DIM // 2]
        a22 = w3[e][:, DIM // 2:]
        am = np.stack([a11 + a22, a21 + a22, a11, a22,
                       a11 + a12, a21 - a11, a12 - a22], 0)  # [7, 1408, 1024]
        apk = np.ascontiguousarray(
            am.reshape(7, N_MT, P, HK, P).transpose(4, 1, 0, 3, 2)
            .astype(NPF16))
        w2f = np.ascontiguousarray(
            (W2F_SCALE * w2[e][:, N_F8F * P:])
            .reshape(N_DT, P, N_W2F, P).transpose(3, 0, 2, 1).astype(NPF16))
        w28 = np.clip(w2[e][:, :(N_F8F + N_F8H) * P] * W28_SCALE, -240.0, 240.0)
        w28 = np.ascontiguousarray(
            w28.reshape(N_DT, P, N_F8F + N_F8H, P).transpose(3, 0, 2, 1)
            .astype(F8))
        maps.append({"ap": apk, "w2p": w2f, "w28p": w28})
    return maps


def kernel(x, w1, w2, w3, num_tokens_per_expert, _trace=False):
    x = np.ascontiguousarray(np.asarray(x, dtype=np.float32))
    w1 = np.ascontiguousarray(np.asarray(w1, dtype=np.float32))
    w2 = np.ascontiguousarray(np.asarray(w2, dtype=np.float32))
    w3 = np.ascontiguousarray(np.asarray(w3, dtype=np.float32))
    counts = np.asarray(num_tokens_per_expert, dtype=np.int64)

    cs = np.cumsum(counts)
    starts = np.minimum(np.concatenate([[0], cs[:-1]]), N_TOKENS)
    ends = np.minimum(cs, N_TOKENS)
    lens = np.maximum(ends - starts, 0)

    wmaps = _pack_weights(w1, w2, w3)
    out = np.zeros((N_TOKENS, DIM), np.float32)
    trace_info = []

    n_passes = max(1, int(np.max(np.ceil(lens / T))))
    for k in range(n_passes):
        in_maps = []
        for e in range(N_EXPERTS):
            s = int(starts[e]) + k * T
            xe = np.zeros((T, DIM), np.float32)
            avail = x[s:s + T]
            if avail.shape[0]:
                xe[:avail.shape[0]] = avail
            xtp = np.ascontiguousarray(
                xe.T.reshape(N_DN, P, T // TSUB, TSUB)
                .transpose(1, 2, 0, 3).astype(NPF16))
            in_maps.append({"xtp": xtp, **wmaps[e]})
        res = run_bass_kernel_spmd(
            _get_nc(), in_maps, list(range(N_CORES)), trace=_trace
        )
        if _trace:
            trace_info.append(res)
        for e in range(N_EXPERTS):
            s = int(starts[e]) + k * T
            cnt = min(int(ends[e]) - s, T)
            if cnt > 0:
                out[s:s + cnt] = res.results[e]["outt"].T[:cnt].astype(np.float32)

    if _trace:
        return out, trace_info
    return out


# revision 29
# speedup vs baseline: 1.1165x; 1.0019x over previous
"""Grouped-experts SwiGLU FFN (MoE) on 8 Trainium2 NeuronCores.

Expert-parallel: core e owns expert e's weights and its contiguous token
slice (tokens are pre-sorted by expert).  Per core, out.T = W2 @ h where
h = silu(W1 x) * (W3 x), x [2048 dim, 2048 tok].

Stage 1 runs one level of Strassen on the stacked [W1; W3] @ x product:
A = [W1; W3] is [2816, 2048] (M-halves of 1408 = 11 clean 128-row tiles),
K = 2048 and N = 2048 both split 1024.  7 M-products instead of 8
block-products cuts stage-1 PE streaming by 12.5% (1232 vs 1408 matmuls).
The A-side combinations are folded into host weight packing; the B-side
(x) sums are 5 DVE adds per token-column-pair; M-product drains and the
C recombination ride the otherwise-idle Vector engine under the PE
shadow.  silu(C_top) * C_bot then feeds stage 2 unchanged.

Precision: fp16 operands everywhere (same PE speed as bf16, 8x lower
noise), plus e4m3 fp8 DoubleRow (2 contraction blocks per matmul) for
stage-2 h-blocks 0,1 on all tokens and blocks 2,3 on tokens 0:1024.
Offline-simulated rel err 1.958e-2 vs the 2e-2 budget (HW measured
tracks the simulator within ~3e-4).  fp16 w2 is host-scaled x128 so its
PSUM partials match the (4h)(32w2) fp8 partials; drains scale by 1/128.

Head: the runtime preamble is ~7us; a few junk warm-up matmuls issued
first get the PE HAM clock-gate to 2.4 GHz before real operands land,
and the DMA descriptor order streams exactly what the first matmul
chain needs (A(m2) tiles + x chunk 0) at full bandwidth.
"""

import numpy as np
import ml_dtypes

import concourse.bass as bass
from concourse import bacc
import concourse.mybir as mybir
from concourse.tile import TileContext
from concourse.bass_utils import run_bass_kernel_spmd

N_TOKENS = 16384
DIM = 2048
HIDDEN = 1408
N_EXPERTS = 8
N_CORES = 8

P = 128
T = 2048                 # token capacity per core per pass
N_DN = DIM // P          # 16 k-blocks (full K)
HK = 8                   # k-blocks per Strassen half (1024/128)
N_MT = HIDDEN // P       # 11 output-row tiles per M-product
N_HT = N_MT              # 11 h blocks
N_F8F = 2                # h-blocks 0,1: fp8 DR for all tokens
N_F8H = 2                # h-blocks 2,3: fp8 DR for tokens 0:1024
N_HB = 7                 # h-blocks 4..10: fp16-only path
N_W2F = 9                # fp16 w2 blocks 2..10 (2,3 used for tokens 1024:2048)
N_DT = DIM // P          # 16 output-row tiles in stage 2
TSUB = 512
HALF = T // 2

F32 = mybir.dt.float32
F16 = mybir.dt.float16
F8E4 = mybir.dt.float8e4
SILU = mybir.ActivationFunctionType.Silu
COPY = mybir.ActivationFunctionType.Copy
DROW = mybir.MatmulPerfMode.DoubleRow
ADD = mybir.AluOpType.add
SUB = mybir.AluOpType.subtract
NPF16 = np.float16
F8 = ml_dtypes.float8_e4m3
H8_SCALE = 4.0           # ht8 = e4m3(4h)
W28_SCALE = 32.0         # fp8 w2 blocks x32 -> PSUM partial 128*out
W2F_SCALE = 128.0        # fp16 w2 x128 -> matches fp8 partial scale

# Strassen M-product index order within phase B (phase A runs m2 alone):
# m4 first (its T-sum needs only x chunk A), then m5 (raw), m1/m3/m6/m7.
M2 = 1                   # m-index (0-based) of M2 = (A21+A22) B11
PHASE_B = [3, 4, 0, 2, 5, 6]   # m4, m5, m1, m3, m6, m7
N_WU = 10                # warm-up matmuls during the runtime preamble


def _build_program() -> bass.Bass:
    nc = bacc.Bacc()
    # A combinations: ap[p, mt, m, kb, c] = A_m[mt*128+c, kb*128+p]
    ap = nc.declare_dram_parameter("ap", [P, N_MT, 7, HK, P], F16, isOutput=False)
    xtp = nc.declare_dram_parameter(
        "xtp", [P, T // TSUB, N_DN, TSUB], F16, isOutput=False)
    w2p = nc.declare_dram_parameter("w2p", [P, N_DT, N_W2F, P], F16, isOutput=False)
    w28p = nc.declare_dram_parameter(
        "w28p", [P, N_DT, N_F8F + N_F8H, P], F8E4, isOutput=False)
    outt = nc.declare_dram_parameter("outt", [DIM, T], F16, isOutput=True)

    with TileContext(nc) as tc:
        with (
            tc.tile_pool(name="wu", bufs=1) as wu_pool,
            tc.tile_pool(name="xt", bufs=1) as xt_pool,
            tc.tile_pool(name="at", bufs=8) as at_pool,
            tc.tile_pool(name="tt", bufs=1) as tt_pool,
            tc.tile_pool(name="m2s", bufs=1) as m2s_pool,
            tc.tile_pool(name="ms", bufs=2) as ms_pool,
            tc.tile_pool(name="cc", bufs=1) as cc_pool,
            tc.tile_pool(name="ht", bufs=1) as ht_pool,
            tc.tile_pool(name="w2", bufs=2) as w2_pool,
            tc.tile_pool(name="w28", bufs=2) as w28_pool,
            tc.tile_pool(name="tmp", bufs=4) as tmp_pool,
            tc.tile_pool(name="ob", bufs=3) as ob_pool,
            tc.tile_pool(name="ps", bufs=1, space="PSUM") as ps_pool,
        ):
            # ---- PE warm-up: junk matmuls issued before any DMA lands so
            # the HAM clock-gate reaches 2.4 GHz during the ~7us preamble.
            # memset on gpsimd: its sequencer is ready ~2us before vector's.
            wu = wu_pool.tile([P, TSUB], F16)
            nc.gpsimd.memset(wu[:], 0.0)
            for _ in range(N_WU):
                psw = ps_pool.tile([P, TSUB], F32, bufs=4, name="psm")
                nc.tensor.matmul(psw[:], lhsT=wu[:, 0:P], rhs=wu[:],
                                 start=True, stop=True)

            # ---- bulk input tiles: chunk pair (cA, cB) per column pass;
            # p=1 reuses p=0's buffers (WAR clears once p=0 stops reading)
            # cA is split into three tiles: dependency tracking is per-tile,
            # so the opening matmul chain only waits on the 512KB it needs.
            def chunk_tiles():
                cAlo1 = xt_pool.tile([P, HK // 2, TSUB], F16, bufs=1,
                                     tag="xta1", name="xcAlo1")
                cAlo2 = xt_pool.tile([P, HK // 2, TSUB], F16, bufs=1,
                                     tag="xta2", name="xcAlo2")
                cAhi = xt_pool.tile([P, HK, TSUB], F16, bufs=1,
                                    tag="xth", name="xcAhi")
                cB = xt_pool.tile([P, N_DN, TSUB], F16, bufs=1, tag="xtb",
                                  name="xcB")
                return cAlo1, cAlo2, cAhi, cB

            def load_a(mt, m):
                at = at_pool.tile([P, HK, P], F16, tag="at")
                nc.sync.dma_start(out=at[:], in_=ap[:, mt, m, :, :])
                return at

            # Head-critical DMA order: A(mt0,m2), x chunk0 (fine-grained so
            # the first matmul chain starts ASAP), more A(m2), x chunk2.
            # Head-critical descriptors only: A2(mt0) + B11 of chunk 0 feed
            # the opening matmul chains.  Everything else (cAhi, chunk 2)
            # is emitted after phase A's A-tile loads so it cannot steal
            # bandwidth from them.
            cAlo1_0, cAlo2_0, cAhi0, cB0 = chunk_tiles()
            a2_tiles = {}
            a2_tiles[0] = load_a(0, M2)
            nc.sync.dma_start(out=cAlo1_0[:], in_=xtp[:, 0, 0:HK // 2, :])
            a2_tiles[1] = load_a(1, M2)
            nc.sync.dma_start(out=cAlo2_0[:], in_=xtp[:, 0, HK // 2:HK, :])
            a2_tiles[2] = load_a(2, M2)
            a2_tiles[3] = load_a(3, M2)
            a2_tiles[4] = load_a(4, M2)

            # ---- stage-1 outputs
            ht = ht_pool.tile([P, N_HB, T], F16)                 # blocks 4..10
            ht23 = ht_pool.tile([P, N_F8H, HALF], F16, name="ht23")  # 2,3 hi-half
            ht8f = ht_pool.tile([P, N_F8F, T], F8E4, name="ht8f")    # 0,1 all
            ht8h = ht_pool.tile([P, N_F8H, HALF], F8E4, name="ht8h")  # 2,3 lo-half

            # ---- stage 1: two column-pair passes (p=0: chunks 0/2, p=1: 1/3)
            for p in range(2):
                if p == 0:
                    cAlo1, cAlo2, cAhi, cB = cAlo1_0, cAlo2_0, cAhi0, cB0
                else:
                    cAlo1, cAlo2, cAhi, cB = chunk_tiles()
                    nc.sync.dma_start(out=cAlo1[:],
                                      in_=xtp[:, 1, 0:HK // 2, :])
                    nc.sync.dma_start(out=cAlo2[:],
                                      in_=xtp[:, 1, HK // 2:HK, :])
                    nc.sync.dma_start(out=cAhi[:], in_=xtp[:, 1, HK:, :])
                m2st = m2s_pool.tile([P, N_MT, TSUB], F16, tag="m2s")
                tt = tt_pool.tile([P, 5, HK, TSUB], F16, tag="tt")

                def calo(kb):
                    return (cAlo1[:, kb, :] if kb < HK // 2
                            else cAlo2[:, kb - HK // 2, :])

                # phase A: M2 = (A21+A22) @ B11 for all mt (no DVE deps)
                for mt in range(N_MT):
                    if p == 0 and mt in a2_tiles:
                        at = a2_tiles[mt]
                    else:
                        at = load_a(mt, M2)
                    psm = ps_pool.tile([P, TSUB], F32, bufs=4, name="psm")
                    for kb in range(HK):
                        nc.tensor.matmul(psm[:], lhsT=at[:, kb, :],
                                         rhs=calo(kb),
                                         start=(kb == 0), stop=(kb == HK - 1))
                    # drain on the (idle) scalar engine -- keeps DVE free
                    nc.scalar.activation(m2st[:, mt, :], psm[:], COPY)

                # bulk loads for this pass, behind the phase-A A-tiles
                if p == 0:
                    nc.sync.dma_start(out=cAhi[:], in_=xtp[:, 0, HK:, :])
                    for h in range(2):
                        nc.sync.dma_start(out=cB[:, h * HK:(h + 1) * HK, :],
                                          in_=xtp[:, 2, h * HK:(h + 1) * HK, :])
                else:
                    # chunk 3 into cB (WAR: after p=0's last B22 matmul)
                    nc.sync.dma_start(out=cB[:], in_=xtp[:, 3, :, :])
                # T sums (vector FIFO is otherwise idle during phase A, so
                # these run as soon as their chunks land; phase B's first
                # product needs T4)
                # T4 = B21 - B11
                nc.vector.tensor_tensor(tt[:, 1, 0:HK // 2, :],
                                        cAhi[:, 0:HK // 2, :], cAlo1[:], SUB)
                nc.vector.tensor_tensor(tt[:, 1, HK // 2:, :],
                                        cAhi[:, HK // 2:, :], cAlo2[:], SUB)
                # T1 = B11 + B22
                nc.vector.tensor_tensor(tt[:, 0, 0:HK // 2, :], cAlo1[:],
                                        cB[:, HK:HK + HK // 2, :], ADD)
                nc.vector.tensor_tensor(tt[:, 0, HK // 2:, :], cAlo2[:],
                                        cB[:, HK + HK // 2:, :], ADD)
                # T3 = B12 - B22
                nc.vector.tensor_tensor(tt[:, 2, :, :], cB[:, 0:HK, :],
                                        cB[:, HK:, :], SUB)
                # T6 = B11 + B12
                nc.vector.tensor_tensor(tt[:, 3, 0:HK // 2, :], cAlo1[:],
                                        cB[:, 0:HK // 2, :], ADD)
                nc.vector.tensor_tensor(tt[:, 3, HK // 2:, :], cAlo2[:],
                                        cB[:, HK // 2:HK, :], ADD)
                # T7 = B21 + B22
                nc.vector.tensor_tensor(tt[:, 4, :, :], cAhi[:],
                                        cB[:, HK:, :], ADD)

                rhs_by_m = {
                    0: tt[:, 0, :, :],      # M1: T1
                    2: tt[:, 2, :, :],      # M3: T3
                    3: tt[:, 1, :, :],      # M4: T4
                    4: cB[:, HK:, :],       # M5: B22 raw
                    5: tt[:, 3, :, :],      # M6: T6
                    6: tt[:, 4, :, :],      # M7: T7
                }

                # phase B: remaining 6 products per mt + recombine + swiglu.
                # M4/M5/M1/M3 drain to SBUF via scalar; M6/M7 stay in PSUM
                # and are consumed directly by the recombination adds.
                for mt in range(N_MT):
                    mts = ms_pool.tile([P, 4, TSUB], F16, tag="ms")
                    ps_keep = {}
                    for j, m in enumerate(PHASE_B):
                        at = load_a(mt, m)
                        psm = ps_pool.tile([P, TSUB], F32, bufs=4, name="psm")
                        rhs = rhs_by_m[m]
                        for kb in range(HK):
                            nc.tensor.matmul(psm[:], lhsT=at[:, kb, :],
                                             rhs=rhs[:, kb, :],
                                             start=(kb == 0),
                                             stop=(kb == HK - 1))
                        if j < 4:
                            nc.scalar.activation(mts[:, j, :], psm[:], COPY)
                        else:
                            ps_keep[m] = psm
                    m1 = mts[:, 2, :]
                    m3 = mts[:, 3, :]
                    m4 = mts[:, 0, :]
                    m5 = mts[:, 1, :]
                    m6 = ps_keep[5][:]
                    m7 = ps_keep[6][:]
                    m2 = m2st[:, mt, :]
                    cc = cc_pool.tile([P, 6, TSUB], F32, tag="cc")
                    c11, c12, c21, c22, s0, s1 = (cc[:, i, :] for i in range(6))
                    # C11 = M1 + M4 - M5 + M7  (no in-place DVE ops)
                    nc.vector.tensor_tensor(s0, m1, m4, ADD)
                    nc.vector.tensor_tensor(s1, s0, m5, SUB)
                    nc.vector.tensor_tensor(c11, s1, m7, ADD)
                    # C21 = M2 + M4
                    nc.vector.tensor_tensor(c21, m2, m4, ADD)
                    # C12 = M3 + M5
                    nc.vector.tensor_tensor(c12, m3, m5, ADD)
                    # C22 = M1 - M2 + M3 + M6
                    nc.vector.tensor_tensor(s0, m1, m2, SUB)
                    nc.vector.tensor_tensor(s1, s0, m3, ADD)
                    nc.vector.tensor_tensor(c22, s1, m6, ADD)

                    lo = p * TSUB            # token cols within each half
                    # half 1 (tokens 0:1024): fp8 for mt<4, fp16 otherwise
                    tmp = tmp_pool.tile([P, TSUB], F32, tag="tmp")
                    nc.scalar.activation(tmp[:], c11, SILU)
                    if mt < 2:
                        bsc = tmp_pool.tile([P, TSUB], F32, tag="tmp")
                        nc.scalar.activation(bsc[:], c21, COPY, scale=H8_SCALE)
                        nc.vector.tensor_tensor(
                            ht8f[:, mt, lo:lo + TSUB], tmp[:], bsc[:],
                            mybir.AluOpType.mult)
                    elif mt < 4:
                        bsc = tmp_pool.tile([P, TSUB], F32, tag="tmp")
                        nc.scalar.activation(bsc[:], c21, COPY, scale=H8_SCALE)
                        nc.vector.tensor_tensor(
                            ht8h[:, mt - 2, lo:lo + TSUB], tmp[:], bsc[:],
                            mybir.AluOpType.mult)
                    else:
                        nc.vector.tensor_tensor(
                            ht[:, mt - 4, lo:lo + TSUB], tmp[:], c21,
                            mybir.AluOpType.mult)
                    # half 2 (tokens 1024:2048): fp8 only for mt<2
                    tmp2 = tmp_pool.tile([P, TSUB], F32, tag="tmp")
                    nc.scalar.activation(tmp2[:], c12, SILU)
                    if mt < 2:
                        bsc2 = tmp_pool.tile([P, TSUB], F32, tag="tmp")
                        nc.scalar.activation(bsc2[:], c22, COPY, scale=H8_SCALE)
                        nc.vector.tensor_tensor(
                            ht8f[:, mt, HALF + lo:HALF + lo + TSUB],
                            tmp2[:], bsc2[:], mybir.AluOpType.mult)
                    elif mt < 4:
                        nc.vector.tensor_tensor(
                            ht23[:, mt - 2, lo:lo + TSUB], tmp2[:], c22,
                            mybir.AluOpType.mult)
                    else:
                        nc.vector.tensor_tensor(
                            ht[:, mt - 4, HALF + lo:HALF + lo + TSUB],
                            tmp2[:], c22, mybir.AluOpType.mult)

            # ---- stage 2: OUT.T[d, t] = sum_h W2T[h, d] * HT[h, t]
            NTS = T // TSUB
            for idt in range(N_DT):
                w2i = w2_pool.tile([P, N_W2F, P], F16, tag="w2")
                nc.sync.dma_start(out=w2i[:], in_=w2p[:, idt, :, :])
                w28i = w28_pool.tile([P, N_F8F + N_F8H, P], F8E4, tag="w28")
                nc.sync.dma_start(out=w28i[:], in_=w28p[:, idt, :, :])
                ob = ob_pool.tile([P, T], F16)
                for its in range(NTS):
                    last = (idt == N_DT - 1 and its == NTS - 1)
                    # the final token chunk runs as two half-width PSUM
                    # groups so only 256 columns of work remain after the
                    # very last matmul; the two drains ride scalar+vector
                    # and sync+gpsimd queues in parallel
                    halves = (((0, TSUB),) if not last
                              else ((0, TSUB // 2), (TSUB // 2, TSUB)))
                    for hv, (c0, c1) in enumerate(halves):
                        seg = slice(its * TSUB + c0, its * TSUB + c1)
                        w = c1 - c0
                        pso = ps_pool.tile([P, TSUB], F32, bufs=4, name="pso")
                        po = pso[:, 0:w]
                        nc.tensor.matmul(po, lhsT=w28i[:, 0:2, :],
                                         rhs=ht8f[:, :, seg], start=True,
                                         stop=False, perf_mode=DROW)
                        if its < 2:
                            nc.tensor.matmul(po, lhsT=w28i[:, 2:4, :],
                                             rhs=ht8h[:, :, seg], start=False,
                                             stop=False, perf_mode=DROW)
                        else:
                            hseg = slice(seg.start - HALF, seg.stop - HALF)
                            for hb in range(N_F8H):
                                nc.tensor.matmul(po, lhsT=w2i[:, hb, :],
                                                 rhs=ht23[:, hb, hseg],
                                                 start=False, stop=False)
                        for j in range(N_HB):
                            nc.tensor.matmul(po, lhsT=w2i[:, N_F8H + j, :],
                                             rhs=ht[:, j, seg], start=False,
                                             stop=(j == N_HB - 1))
                        if not last:
                            nc.scalar.activation(ob[:, seg], po, COPY,
                                                 scale=1.0 / W2F_SCALE)
                            if idt == N_DT - 1:
                                nc.sync.dma_start(
                                    out=outt[idt * P:(idt + 1) * P, seg],
                                    in_=ob[:, seg])
                        elif hv == 0:
                            nc.scalar.activation(ob[:, seg], po, COPY,
                                                 scale=1.0 / W2F_SCALE)
                            nc.sync.dma_start(
                                out=outt[idt * P:(idt + 1) * P, seg],
                                in_=ob[:, seg])
                        else:
                            nc.vector.tensor_scalar_mul(ob[:, seg], po,
                                                        1.0 / W2F_SCALE)
                            nc.gpsimd.dma_start(
                                out=outt[idt * P:(idt + 1) * P, seg],
                                in_=ob[:, seg])
                if idt < N_DT - 1:
                    nc.sync.dma_start(out=outt[idt * P:(idt + 1) * P, :],
                                      in_=ob[:])
    nc.compile()
    return nc


_CACHE: dict = {}


def _get_nc() -> bass.Bass:
    if "nc" not in _CACHE:
        _CACHE["nc"] = _build_program()
    return _CACHE["nc"]


def _pack_weights(w1, w2, w3):
    maps = []
    for e in range(N_EXPERTS):
        a11 = w1[e][:, :DIM // 2]
        a12 = w1[e][:, DIM // 2:]
        a21 = w3[e][:, :DIM // 2]
        a22 = w3[e][:, DIM // 2:]
        am = np.stack([a11 + a22, a21 + a22, a11, a22,
                       a11 + a12, a21 - a11, a12 - a22], 0)  # [7, 1408, 1024]
        apk = np.ascontiguousarray(
            am.reshape(7, N_MT, P, HK, P).transpose(4, 1, 0, 3, 2)
            .astype(NPF16))
        w2f = np.ascontiguousarray(
            (W2F_SCALE * w2[e][:, N_F8F * P:])
            .reshape(N_DT, P, N_W2F, P).transpose(3, 0, 2, 1).astype(NPF16))
        w28 = np.clip(w2[e][:, :(N_F8F + N_F8H) * P] * W28_SCALE, -240.0, 240.0)
        w28 = np.ascontiguousarray(
            w28.reshape(N_DT, P, N_F8F + N_F8H, P).transpose(3, 0, 2, 1)
            .astype(F8))
        maps.append({"ap": apk, "w2p": w2f, "w28p": w28})
    return maps


def kernel(x, w1, w2, w3, num_tokens_per_expert, _trace=False):
    x = np.ascontiguousarray(np.asarray(x, dtype=np.float32))
    w1 = np.ascontiguousarray(np.asarray(w1, dtype=np.float32))
    w2 = np.ascontiguousarray(np.asarray(w2, dtype=np.float32))
    w3 = np.ascontiguousarray(np.asarray(w3, dtype=np.float32))
    counts = np.asarray(num_tokens_per_expert, dtype=np.int64)

    cs = np.cumsum(counts)
    starts = np.minimum(np.concatenate([[0], cs[:-1]]), N_TOKENS)
    ends = np.minimum(cs, N_TOKENS)
    lens = np.maximum(ends - starts, 0)

    wmaps = _pack_weights(w1, w2, w3)
    out = np.zeros((N_TOKENS, DIM), np.float32)
    trace_info = []

    n_passes = max(1, int(np.max(np.ceil(lens / T))))
    for k in range(n_passes):
        in_maps = []
        for e in range(N_EXPERTS):
            s = int(starts[e]) + k * T
            xe = np.zeros((T, DIM), np.float32)
            avail = x[s:s + T]
            if avail.shape[0]:
                xe[:avail.shape[0]] = avail
            xtp = np.ascontiguousarray(
                xe.T.reshape(N_DN, P, T // TSUB, TSUB)
                .transpose(1, 2, 0, 3).astype(NPF16))
            in_maps.append({"xtp": xtp, **wmaps[e]})
        res = run_bass_kernel_spmd(
            _get_nc(), in_maps, list(range(N_CORES)), trace=_trace
        )
        if _trace:
            trace_info.append(res)
        for e in range(N_EXPERTS):
            s = int(starts[e]) + k * T
            cnt = min(int(ends[e]) - s, T)
            if cnt > 0:
                out[s:s + cnt] = res.results[e]["outt"].T[:cnt].astype(np.float32)

    if _trace:
        return out, trace_info
    return out
